# revision 1
# baseline (speedup 1.0000x reference)
"""Trainium2 Bass kernel for nn_Attention_Module_15152644620833 (v3).

Reference computation (T=4096, B=8, D=1024, H=64, half=2048):
    q   = x[:half] @ Wq + bq            (half, B, H)
    k   = x @ Wk + bk                   (T, B, H)
    val = x @ Wv + bv                   (T, B, H)
    r   = posenc(T, D) @ Wr + br        (T, H)
    scores[b] = q[b] @ (k[b] + r).T + bias[b][None, :]
        where bias[b][m] = sum(u) * k[m,b,:].sum() + sum(v) * r[m,:].sum()
    causal mask on first `half` key positions, softmax over all T keys,
    out = attn @ val                    (half, B, H)

Sharding: data-parallel over batch, one batch per NeuronCore (8 cores).
Each core receives its batch slice pre-transposed (x.T, contiguous).  The
positional-encoding projection r (identical on all cores) is sharded: each
core computes a 512-key slice of r.T and the full r.T is AllGathered.

Per-core device algorithm (f32r matmuls, fp32 PSUM):
    K2 (128, T):  rows 0:64 = k.T + bk, rows 64:128 = r.T + br
    q2 (128, half): rows 0:64 = q.T + bq, rows 64:128 = the same q.T
        -> scoresT(m,t) = K2[:,mtile].T @ q2 = q.k + q.r   (K=128)
    softmax key bias folded multiplicatively into val:
        exp(s + bias[m]) = exp(s)*eb[m]; eb scales both the val columns and
        the ones column (denominator), so attn is unchanged (exact).
        bias[m] = K2[:,m].T @ [u_sum x64; v_sum x64]   (one N=1 matmul/tile)
    causal mask: accumulate identity.T @ maskA (-1e30) into scores PSUM of
        diagonal tiles; fully-masked tiles are skipped.
    expT = exp(scoresT)  (no max subtraction: |scores| < ~60, safe in f32)
    outT (65, 512) += valaug[mtile].T @ expT  per query chunk (col 64 of
        valaug = eb -> row 64 of outT = softmax denominator)
    out (128, 64) = transpose(outT) * (1/denominator)

Schedule: sweep 1 streams x.T and runs all gather-independent projections
while the AllGather is in flight (its DMAs ride the ACT HWDGE ring so they
cannot head-of-line-block the x.T stream on the SP ring); sweep 2 runs
attention query-chunk-outer with the attnval matmuls software-pipelined two
exp-groups behind the score matmuls.
"""

import math

import numpy as np

T, B, D, H = 4096, 8, 1024, 64
HALF = T // 2
P = 128
DC = D // P          # 8 d-chunks
NCH = T // 512       # 8 key chunks of 512
NTQ = HALF // 512    # 4 query chunks of 512
MT = T // P          # 32 key tiles of 128
NCORES = 8

_CACHE = {}


def _posenc_T():
    """Constant positional encoding, transposed to (D, T), float32."""
    pos = np.arange(T, dtype=np.float32)[:, None]
    div = np.exp(
        (np.arange(0, D, 2, dtype=np.float32)
         * np.float32(-(math.log(10000.0) / D))).astype(np.float32)
    ).astype(np.float32)
    ang = (pos * div).astype(np.float32)
    pe = np.stack([np.sin(ang), np.cos(ang)], axis=-1).reshape(T, D)
    return np.ascontiguousarray(pe.astype(np.float32).T)


def _live(tq, mt):
    """Key tile mt contributes to query chunk tq (not fully masked)."""
    m0 = mt * P
    return not (m0 >= tq * 512 + 512 and m0 < HALF)


def _is_diag(tq, mt):
    return tq * 512 <= mt * P < tq * 512 + 512


def _build_module():
    import concourse.bacc as bacc
    import concourse.bass_isa as bass_isa
    import concourse.mybir as mybir
    from concourse.masks import make_identity
    from concourse.tile import TileContext

    f32 = mybir.dt.float32
    f32r = mybir.dt.float32r
    Exp = mybir.ActivationFunctionType.Exp

    nc = bacc.Bacc(num_devices=NCORES)

    xT_h = nc.dram_tensor("xT", [D, T], f32r, kind="ExternalInput")
    peTs_h = nc.dram_tensor("peTs", [D, 512], f32r, kind="ExternalInput")
    wkv_h = nc.dram_tensor("wkv", [D, 2 * H], f32r, kind="ExternalInput")
    wqq_h = nc.dram_tensor("wqq", [D, 2 * H], f32r, kind="ExternalInput")
    wr2_h = nc.dram_tensor("wr2", [D, 2 * H], f32r, kind="ExternalInput")
    bkv_h = nc.dram_tensor("bkv", [2 * H, 1], f32, kind="ExternalInput")
    bqq_h = nc.dram_tensor("bqq", [2 * H, 1], f32, kind="ExternalInput")
    br_h = nc.dram_tensor("br", [H, 1], f32, kind="ExternalInput")
    u_h = nc.dram_tensor("u", [H, 1], f32, kind="ExternalInput")
    v_h = nc.dram_tensor("v", [H, 1], f32, kind="ExternalInput")
    out_h = nc.dram_tensor("out", [HALF, H], f32, kind="ExternalOutput")

    xT_r = xT_h[:, :].rearrange("(c p) t -> p c t", p=P)       # (128, 8, T)
    peTs_r = peTs_h[:, :].rearrange("(c p) t -> p c t", p=P)   # (128, 8, 512)
    wkv_r = wkv_h[:, :].rearrange("(c p) h -> p c h", p=P)
    wqq_r = wqq_h[:, :].rearrange("(c p) h -> p c h", p=P)
    wr2_r = wr2_h[:, :].rearrange("(c p) h -> p c h", p=P)
    out_r = out_h[:, :].rearrange("(g p) h -> p g h", p=P)     # (128, 16, 64)

    with TileContext(nc) as tc, tc.tile_pool(name="persist", bufs=1) as persist:

        def _tile(shape, name, dt=f32):
            return persist.tile(shape, dt, name=name)

        # ---- persistent SBUF tiles -------------------------------------
        wkv_sb = _tile([P, DC, 2 * H], "wkv_sb", f32r)
        wqq_sb = _tile([P, DC, 2 * H], "wqq_sb", f32r)
        wr2_sb = _tile([P, DC, 2 * H], "wr2_sb", f32r)
        bkv_sb = _tile([2 * H, 1], "bkv_sb")
        bqq_sb = _tile([2 * H, 1], "bqq_sb")
        brc_sb = _tile([2 * H, 1], "brc_sb")    # br loaded at rows 64:128
        u_cl = _tile([H, 1], "u_cl")
        v_cl = _tile([H, 1], "v_cl")
        u_all = _tile([H, 1], "u_all")
        v_all = _tile([H, 1], "v_all")
        uvf = _tile([2 * H, 1], "uvf")
        uv_col = _tile([2 * H, 4], "uv_col", f32r)
        id_sb = _tile([P, P], "id_sb", f32r)
        maskA = _tile([P, 4, 512], "maskA", f32r)
        K2 = _tile([P, T], "K2", f32r)          # 0:64 k.T+bk, 64:128 r.T+br
        vT = _tile([P, T], "vT", f32r)          # rows 64:128 = v.T+bv
        q2T = _tile([P, HALF], "q2T", f32r)     # rows 0:64 and 64:128 = q.T
        valaug = _tile([P, MT, H + 1], "valaug", f32r)
        ebias = _tile([P, MT], "ebias")
        outall = _tile([P, HALF // P, H], "outall")

        # ---- constants / small setup -----------------------------------
        nc.sync.dma_start(wr2_sb[:], wr2_r)
        nc.sync.dma_start(wkv_sb[:], wkv_r)
        nc.sync.dma_start(wqq_sb[:], wqq_r)
        nc.sync.dma_start(bkv_sb[:], bkv_h[:, :])
        nc.sync.dma_start(bqq_sb[:], bqq_h[:, :])
        nc.sync.dma_start(brc_sb[H : 2 * H, :], br_h[:, :])
        nc.sync.dma_start(u_cl[:], u_h[:, :])
        nc.sync.dma_start(v_cl[:], v_h[:, :])

        with (
            tc.tile_pool(name="xstream", bufs=2) as xpool,
            tc.tile_pool(name="dramp", bufs=1, space="DRAM") as dramp,
        ):
            # ---- r.T shard + AllGather kickoff (identical r everywhere) -
            with tc.tile_pool(name="ppj", bufs=1, space="PSUM") as ppj:
                pet = xpool.tile([P, DC, 512], f32r, name="pet", tag="xt")
                nc.sync.dma_start(pet[:], peTs_r)
                rp = ppj.tile([P, 512], f32, name="rp", tag="kv", bufs=2)
                for dc in range(DC):
                    nc.tensor.matmul(
                        rp[:], wr2_sb[:, dc, :], pet[:, dc, :],
                        start=(dc == 0), stop=(dc == DC - 1),
                    )
                rloc_sb = xpool.tile([P, 512], f32r, name="rloc_sb", tag="rloc")
                nc.vector.tensor_scalar_add(
                    rloc_sb[H:P, :], rp[H:P, :], brc_sb[H : 2 * H, :]
                )
                rloc_dr = dramp.tile([H, 512], f32r, name="rloc_dr")
                nc.scalar.dma_start(rloc_dr[:], rloc_sb[H:P, :])
                rTg_dr = dramp.tile([NCORES * H, 512], f32r, name="rTg_dr",
                                    addr_space="Shared")
                nc.gpsimd.collective_compute(
                    "AllGather", mybir.AluOpType.bypass,
                    replica_groups=[list(range(NCORES))],
                    ins=[rloc_dr[:]], outs=[rTg_dr[:]],
                )
                # on the ACT HWDGE ring: this DMA waits on the collective,
                # and on the SP ring it would head-of-line-block the x.T
                # chunk stream behind it
                nc.scalar.dma_start(
                    K2[H:P, :].rearrange("h (c m) -> h c m", c=NCH),
                    rTg_dr[:].rearrange("(c h) m -> h c m", h=H),
                )

                # f32r tiles cannot be produced by memset/affine_select directly
                # (ISA/verifier); build constants in f32 scratch, cast-copy on DVE.
                with tc.tile_pool(name="setupf", bufs=1) as setupf:
                    idf = setupf.tile([P, P], f32, name="idf")
                    make_identity(nc, idf[:])
                    nc.vector.tensor_copy(id_sb[:], idf[:])
                    maskAf = setupf.tile([P, 4, 512], f32, name="maskAf")
                    nc.gpsimd.memset(maskAf[:], 0.0)
                    for rel in range(4):
                        nc.gpsimd.affine_select(
                            out=maskAf[:, rel, :], in_=maskAf[:, rel, :],
                            compare_op=mybir.AluOpType.is_ge, fill=-1e30,
                            base=-P * rel, pattern=[[1, 512]], channel_multiplier=-1,
                        )
                    nc.vector.tensor_copy(maskA[:], maskAf[:])
                    onesf = setupf.tile([P, MT], f32, name="onesf")
                    nc.gpsimd.memset(onesf[:], 1.0)
                    nc.vector.tensor_copy(valaug[:, :, H], onesf[:, 0:MT])

                # sum(u) broadcast to rows 0:64, sum(v) to rows 64:128 of uv_col
                nc.gpsimd.partition_all_reduce(u_all[:], u_cl[:], H, bass_isa.ReduceOp.add)
                nc.gpsimd.partition_all_reduce(v_all[:], v_cl[:], H, bass_isa.ReduceOp.add)
                nc.vector.tensor_copy(uvf[0:H, :], u_all[:])
                nc.sync.dma_start(uvf[H : 2 * H, :], v_all[:])  # partition shift
                nc.vector.tensor_copy(uv_col[:], uvf[:, 0:1].to_broadcast((2 * H, 4)))


                # ---- sweep 1: gather-independent projections ------------
                for c in range(NCH):
                    sl = slice(c * 512, (c + 1) * 512)
                    xt = xpool.tile([P, DC, 512], f32r, name="xt", tag="xt")
                    nc.sync.dma_start(xt[:], xT_r[:, :, sl])

                    kvp = ppj.tile([P, 512], f32, name="kvp", tag="kv", bufs=2)
                    for dc in range(DC):
                        nc.tensor.matmul(
                            kvp[:], wkv_sb[:, dc, :], xt[:, dc, :],
                            start=(dc == 0), stop=(dc == DC - 1),
                        )
                    nc.vector.tensor_scalar_add(
                        K2[0:H, sl], kvp[0:H, :], bkv_sb[0:H, :]
                    )
                    nc.vector.tensor_scalar_add(
                        vT[H:P, sl], kvp[H:P, :], bkv_sb[H : 2 * H, :]
                    )

                    if c < NTQ:
                        qp = ppj.tile([P, 512], f32, name="qp", tag="kv", bufs=2)
                        for dc in range(DC):
                            nc.tensor.matmul(
                                qp[:], wqq_sb[:, dc, :], xt[:, dc, :],
                                start=(dc == 0), stop=(dc == DC - 1),
                            )
                        nc.vector.tensor_scalar_add(q2T[:, sl], qp[:], bqq_sb[:])

                    for j in range(4):
                        mt = c * 4 + j
                        msl = slice(mt * P, (mt + 1) * P)
                        vp = ppj.tile([P, 512], f32r, name="vp", tag="kv",
                                      bufs=2)[:, 0:H]
                        nc.tensor.transpose(vp[:], vT[H:P, msl], id_sb[H:P, H:P])
                        nc.vector.tensor_copy(valaug[:, mt, 0:H], vp[:])

                # ---- key bias -> eb, folded into valaug (gather-gated) --
                for c in range(NCH):
                    bp = ppj.tile([P, 512], f32, name="bp", tag="kv",
                                  bufs=2)[:, 0:16]
                    for j in range(4):
                        mt = c * 4 + j
                        msl = slice(mt * P, (mt + 1) * P)
                        nc.tensor.matmul(
                            bp[:, 4 * j : 4 * j + 4], K2[:, msl], uv_col[:],
                            start=True, stop=True,
                        )
                    nc.scalar.activation(
                        ebias[:, c * 4 : (c + 1) * 4], bp[:, 0:16:4], Exp
                    )
                    for j in range(4):
                        mt = c * 4 + j
                        nc.vector.tensor_scalar_mul(
                            valaug[:, mt, :], valaug[:, mt, :],
                            ebias[:, mt : mt + 1],
                        )

            # ---- sweep 2: attention, query-chunk outer ------------------
            with (
                tc.tile_pool(name="expp", bufs=4) as exp_pool,
                tc.tile_pool(name="posb", bufs=2) as osb_pool,
                tc.tile_pool(name="pinv", bufs=2) as inv_pool,
                tc.tile_pool(name="ps_s", bufs=2, space="PSUM") as pp_s,
                tc.tile_pool(name="ps_o", bufs=2, space="PSUM") as pp_o,
            ):
                for tq in range(NTQ):
                    tsl = slice(tq * 512, (tq + 1) * 512)
                    mts = [mt for mt in range(MT) if _live(tq, mt)]
                    groups = [mts[i : i + 3] for i in range(0, len(mts), 3)]
                    oT_ps = pp_o.tile([H + 1, 512], f32, name="oT_ps")
                    n_done = 0
                    pend = []
                    for g in groups + [None, None]:
                        if g is not None:
                            sp = pp_s.tile([P, 3, 512], f32, name="sp", tag="sp")
                            for i, mt in enumerate(g):
                                msl = slice(mt * P, (mt + 1) * P)
                                diag = _is_diag(tq, mt)
                                nc.tensor.matmul(
                                    sp[:, i, :], K2[:, msl], q2T[:, tsl],
                                    start=True, stop=not diag,
                                )
                                if diag:
                                    nc.tensor.matmul(
                                        sp[:, i, :], id_sb[:, :],
                                        maskA[:, mt - tq * 4, :],
                                        start=False, stop=True,
                                    )
                            ex = exp_pool.tile([P, 3, 512], f32r, name="ex")
                            nc.scalar.activation(
                                ex[:, 0 : len(g), :], sp[:, 0 : len(g), :], Exp
                            )
                        # attnval emitted two groups late, so the next two
                        # groups' score matmuls sit ahead of it in the
                        # in-order PE queue and PE never stalls on exp
                        if g is not None:
                            pend.append((g, ex))
                        if (len(pend) > 2) or (g is None and pend):
                            pg, pex = pend.pop(0)
                            for i, mt in enumerate(pg):
                                nc.tensor.matmul(
                                    oT_ps[:], valaug[:, mt, :], pex[:, i, :],
                                    start=(n_done == 0),
                                    stop=(n_done == len(mts) - 1),
                                )
                                n_done += 1
                    oT_sb = osb_pool.tile([H + 1, 512], f32, name="oT_sb")
                    nc.vector.tensor_copy(oT_sb[:], oT_ps[:])
                    for j in range(4):
                        # share the accumulator slots (free once oT_sb is
                        # copied) instead of the score slots, which would
                        # stall the next query chunk's score matmuls
                        tp = pp_o.tile([P, H + 1], f32, name="tp", tag="oT_ps")
                        nc.tensor.transpose(
                            tp[:], oT_sb[:, j * P : (j + 1) * P],
                            id_sb[0 : H + 1, 0 : H + 1].bitcast(f32),
                        )
                        inv = inv_pool.tile([P, 1], f32, name="inv")
                        nc.vector.reciprocal(inv[:], tp[:, H : H + 1])
                        nc.vector.tensor_scalar_mul(
                            outall[:, tq * 4 + j, :], tp[:, 0:H], inv[:]
                        )
                nc.sync.dma_start(out_r, outall[:])

    nc.compile()
    return nc


def _get_module():
    if "nc" not in _CACHE:
        _CACHE["nc"] = _build_module()
    return _CACHE["nc"]


def _make_in_maps(inputs):
    inp = np.asarray(inputs["inp_data"], dtype=np.float32)
    Wq = np.asarray(inputs["Wq"], dtype=np.float32)
    bq = np.asarray(inputs["bq"], dtype=np.float32)
    Wk = np.asarray(inputs["Wk"], dtype=np.float32)
    bk = np.asarray(inputs["bk"], dtype=np.float32)
    Wv = np.asarray(inputs["Wv"], dtype=np.float32)
    bv = np.asarray(inputs["bv"], dtype=np.float32)
    Wr = np.asarray(inputs["Wr"], dtype=np.float32)
    br = np.asarray(inputs["br"], dtype=np.float32)
    u = np.asarray(inputs["u"], dtype=np.float32)
    v = np.asarray(inputs["v"], dtype=np.float32)

    if "peT" not in _CACHE:
        _CACHE["peT"] = _posenc_T()
    peT = _CACHE["peT"]
    common = {
        "wkv": np.ascontiguousarray(np.concatenate([Wk, Wv], axis=1)),
        "wqq": np.ascontiguousarray(np.concatenate([Wq, Wq], axis=1)),
        "wr2": np.ascontiguousarray(
            np.concatenate([np.zeros_like(Wr), Wr], axis=1)
        ),
        "bkv": np.ascontiguousarray(np.concatenate([bk, bv]).reshape(2 * H, 1)),
        "bqq": np.ascontiguousarray(np.concatenate([bq, bq]).reshape(2 * H, 1)),
        "br": np.ascontiguousarray(br.reshape(H, 1)),
        "u": np.ascontiguousarray(u.reshape(H, 1)),
        "v": np.ascontiguousarray(v.reshape(H, 1)),
    }
    in_maps = []
    for b in range(NCORES):
        m = {
            "xT": np.ascontiguousarray(inp[:, b, :].T),
            "peTs": np.ascontiguousarray(peT[:, b * 512 : (b + 1) * 512]),
        }
        m.update(common)
        in_maps.append(m)
    return in_maps


def _run(in_maps, trace=False):
    from concourse.bass_utils import run_bass_kernel_spmd

    nc = _get_module()
    return run_bass_kernel_spmd(
        nc, in_maps, core_ids=list(range(NCORES)), trace=trace
    )


def _timed_run(in_maps, iters=5, reps=1):
    """Replicates bass2jax.run_bass_via_pjrt's multi-core path, but keeps the
    jitted callable and device-resident inputs so repeated executions can be
    wall-clock timed (no NTFF profiling is available through the axon client).
    """
    import time

    import jax
    import concourse.mybir as mybir
    from concourse.bass2jax import (
        _bass_exec_p,
        install_neuronx_cc_hook,
        partition_id_tensor,
    )
    from jax.experimental.shard_map import shard_map
    from jax.sharding import Mesh, NamedSharding, PartitionSpec

    nc = _get_module()
    install_neuronx_cc_hook()
    partition_name = nc.partition_id_tensor.name if nc.partition_id_tensor else None

    in_names, out_names, out_avals, zero_shapes = [], [], [], []
    for alloc in nc.m.functions[0].allocations:
        if not isinstance(alloc, mybir.MemoryLocationSet):
            continue
        name = alloc.memorylocations[0].name
        if alloc.kind == "ExternalInput":
            if name != partition_name:
                in_names.append(name)
        elif alloc.kind == "ExternalOutput":
            out_names.append(name)
            shape = tuple(alloc.tensor_shape)
            dtype = mybir.dt.np(alloc.dtype)
            out_avals.append(jax.core.ShapedArray(shape, dtype))
            zero_shapes.append((shape, dtype))
    n_params = len(in_names)
    all_names = in_names + out_names
    if partition_name is not None:
        all_names = all_names + [partition_name]
    donate = tuple(range(n_params, n_params + len(out_names)))

    def _body(*args):
        operands = list(args)
        if partition_name is not None:
            operands.append(partition_id_tensor())
        outs = _bass_exec_p.bind(
            *operands,
            out_avals=tuple(out_avals),
            in_names=tuple(all_names),
            out_names=tuple(out_names),
            lowering_input_output_aliases=(),
            sim_require_finite=True,
            sim_require_nnan=True,
            nc=nc,
        )
        return tuple(outs)

    devices = jax.devices()[:NCORES]
    mesh = Mesh(np.asarray(devices), ("core",))
    spec = PartitionSpec("core")
    in_specs = (spec,) * (n_params + len(out_names))
    sharded = jax.jit(
        shard_map(
            _body, mesh=mesh, in_specs=in_specs,
            out_specs=(spec,) * len(out_names), check_rep=False,
        ),
        donate_argnums=donate,
        keep_unused=True,
    )
    sharding = NamedSharding(mesh, spec)
    concat_in = [
        jax.device_put(
            np.concatenate([in_maps[c][nm] for c in range(NCORES)], axis=0), sharding
        )
        for nm in in_names
    ]

    def zeros():
        return [
            jax.device_put(np.zeros((NCORES * s[0], *s[1:]), d), sharding)
            for (s, d) in zero_shapes
        ]

    out = sharded(*concat_in, *zeros())
    jax.block_until_ready(out)
    times = []
    for _ in range(iters):
        zs = zeros()
        jax.block_until_ready(zs)
        t0 = time.perf_counter()
        out = sharded(*concat_in, *zs)
        jax.block_until_ready(out)
        times.append(time.perf_counter() - t0)
    results = {
        nm: np.asarray(out[i]).reshape(NCORES, *out_avals[i].shape)
        for i, nm in enumerate(out_names)
    }
    return results, times


def kernel(**inputs) -> np.ndarray:
    in_maps = _make_in_maps(inputs)
    res = _run(in_maps, trace=False)
    out = np.stack([res.results[b]["out"] for b in range(NCORES)], axis=1)
    return np.ascontiguousarray(out.astype(np.float32))



# revision 40
# speedup vs baseline: 1.2680x; 1.2680x over previous
"""Trainium2 Bass kernel for nn_Attention_Module_15152644620833 (v5).

Reference computation (T=4096, B=8, D=1024, H=64, half=2048):
    q   = x[:half] @ Wq + bq            (half, B, H)
    k   = x @ Wk + bk                   (T, B, H)
    val = x @ Wv + bv                   (T, B, H)
    r   = posenc(T, D) @ Wr + br        (T, H)
    scores[b] = q[b] @ (k[b] + r).T + bias[b][None, :]
        where bias[b][m] = sum(u) * k[m,b,:].sum() + sum(v) * r[m,:].sum()
    causal mask on first `half` key positions, softmax over all T keys,
    out = attn @ val                    (half, B, H)

Sharding: data-parallel over batch, one batch element per NeuronCore.

v5 design (single-sweep pipeline, no collective):
  * r is computed locally on every core with zero communication.  Because
    pe[t] is sinusoidal, r.T for key chunk c equals W'_c.T @ pe0.T where
    pe0 is the FIRST 512-row block of the positional encoding and W'_c is
    Wr with its (sin, cos) row pairs rotated by the chunk phase
    phi_i = 512*c*div_i (host-side weight prep, exact identity).  Chunk
    pairs are packed into one 128-wide lhsT so the 8 chunks cost only
    4x8 matmuls; +br rides a K=1 matmul with a ones row.  pe0/W' are fp16
    (|pe|<=1; rel err ~6e-4 -> negligible vs the 2e-2 gate).
  * K2 (128, T) holds r.T on rows 0:64 (pair row 0:64 drains via DVE, the
    odd-chunk half via a partition-shift DMA) and k.T on rows 64:128;
    q2T holds [q; q] so scoresT = K2[:,msl].T @ q2T in one K=128 matmul.
  * key bias sum(u)*k_sum + sum(v)*r_sum folds multiplicatively into the
    val tiles (exp(s+b) = exp(s)*eb; eb scales both the val columns and
    the denominator ones column, so attn is unchanged, exactly).
  * causal mask: exp runs unmasked (|s|<~70 is fp32-safe); masked entries
    of diagonal-tile exp results are zeroed by a Pool affine_select, so
    the PE never runs mask matmuls.
  * attention is fused into the k/v streaming loop: when x chunk c lands,
    its projections run and every (query-chunk, key-tile) pair that just
    became computable is scored immediately.  Scores run in 2-tile groups
    so each activation-engine exp covers 1024 columns (the per-op PSUM
    access penalty is ~143ns, so per-tile exps would make ACT the
    bottleneck).  attnval accumulates per (iter, tq) event in a PSUM bank,
    then DVE-accumulates into an SBUF oT accumulator.  Score -> exp ->
    attnval is software-pipelined 2 groups deep so the in-order PE queue
    never waits on the activation engine.
  * DMA: the SP ring carries only the big loads in critical order
    (wkv, wqq, pe0, wrot, x chunks); all small transfers ride the ACT
    ring (each DMA costs ~650ns of ring-queue time regardless of size).
  * a dozen warmup matmuls on scratch data ramp the PE DVFS pstate to
    full clock before the first real matmul arrives.
"""

import math

import numpy as np

T, B, D, H = 4096, 8, 1024, 64
HALF = T // 2
P = 128
DC = D // P          # 8 d-chunks
NCH = T // 512       # 8 key/x chunks of 512
NTQ = HALF // 512    # 4 query chunks of 512
MT = T // P          # 32 key tiles of 128
NCORES = 8
LAGG = 2             # software-pipeline lag, in 2-tile score groups

_CACHE = {}


def _div_vec():
    return np.exp(
        np.arange(0, D, 2, dtype=np.float64) * (-(math.log(10000.0) / D))
    )


def _pe0T():
    """First 512 rows of the sin/cos positional encoding, transposed
    to (D, 512)."""
    div = _div_vec()
    ang = np.arange(512, dtype=np.float64)[:, None] * div[None, :]
    pe0 = np.stack([np.sin(ang), np.cos(ang)], axis=-1).reshape(512, D)
    return np.ascontiguousarray(pe0.T)


def _rotate_wr(Wr):
    """W'_c for every chunk c, packed in pairs (g, g+4): (D, 512) with
    columns [g*128 : g*128+64] = W'_g and [g*128+64 : (g+1)*128] =
    W'_{g+4}.  The early chunk of each pair lands on PSUM rows 0:64
    (lane-aligned DVE drain, available immediately); the late chunk's
    partition-shift DMA can wait for a crack in the x-stream.

    Identity: r[512c+s, h] = sum_d pe0[s, d] * W'_c[d, h]  (+ br), where
    W'_c rotates each (sin,cos) row pair of Wr by phi_i = 512*c*div_i.
    """
    div = _div_vec()
    Wr = np.asarray(Wr, dtype=np.float64)
    We, Wo = Wr[0::2, :], Wr[1::2, :]

    def _wc(c):
        phi = 512.0 * c * div
        cph, sph = np.cos(phi)[:, None], np.sin(phi)[:, None]
        Wc = np.empty((D, H), dtype=np.float64)
        Wc[0::2, :] = cph * We - sph * Wo
        Wc[1::2, :] = sph * We + cph * Wo
        return Wc

    cols = []
    for g in range(4):
        cols += [_wc(g), _wc(g + 4)]
    return np.ascontiguousarray(np.concatenate(cols, axis=1))


def _attn_events(c):
    """Events for iter c: list of (tq, [(mt, rel), ...]).  rel is the
    diagonal-tile offset (None for fully-unmasked tiles)."""
    if c < NTQ:
        tiles = []
        for kc in range(c):
            tiles += [(4 * kc + j, None) for j in range(4)]
        tiles += [(4 * c + j, j) for j in range(4)]
        return [(c, tiles)]
    return [(tq, [(4 * c + j, None) for j in range(4)]) for tq in range(NTQ)]


def _build_module():
    import concourse.bacc as bacc
    import concourse.bass_isa as bass_isa
    import concourse.mybir as mybir
    from concourse.masks import make_identity
    from concourse.tile import TileContext

    f32 = mybir.dt.float32
    f32r = mybir.dt.float32r
    f16 = mybir.dt.float16
    Exp = mybir.ActivationFunctionType.Exp

    nc = bacc.Bacc(num_devices=NCORES)

    xT_h = nc.dram_tensor("xT", [D, T], f32r, kind="ExternalInput")
    pe0_h = nc.dram_tensor("pe0", [D, 512], f16, kind="ExternalInput")
    wrot_h = nc.dram_tensor("wrot", [D, 512], f16, kind="ExternalInput")
    wkva_h = nc.dram_tensor("wkva", [D, 2 * H], f32r, kind="ExternalInput")
    wkvb_h = nc.dram_tensor("wkvb", [D, 2 * H], f32r, kind="ExternalInput")
    wqq_h = nc.dram_tensor("wqq", [D, 2 * H], f32r, kind="ExternalInput")
    bkv_h = nc.dram_tensor("bkv", [2 * H, 2], f32, kind="ExternalInput")
    bqq_h = nc.dram_tensor("bqq", [2 * H, 1], f32, kind="ExternalInput")
    brr_h = nc.dram_tensor("brr", [1, 2 * H], f16, kind="ExternalInput")
    u_h = nc.dram_tensor("u", [H, 1], f32, kind="ExternalInput")
    v_h = nc.dram_tensor("v", [H, 1], f32, kind="ExternalInput")
    out_h = nc.dram_tensor("out", [HALF, H], f32, kind="ExternalOutput")

    xT_r = xT_h[:, :].rearrange("(c p) t -> p c t", p=P)       # (128, 8, T)
    pe0_r = pe0_h[:, :].rearrange("(c p) s -> p c s", p=P)     # (128, 8, 512)
    wrot_r = wrot_h[:, :].rearrange("(c p) h -> p c h", p=P)
    wkva_r = wkva_h[:, :].rearrange("(c p) h -> p c h", p=P)
    wkvb_r = wkvb_h[:, :].rearrange("(c p) h -> p c h", p=P)
    wqq_r = wqq_h[:, :].rearrange("(c p) h -> p c h", p=P)
    out_r = out_h[:, :].rearrange("(g p) h -> p g h", p=P)     # (128, 16, 64)

    with TileContext(nc) as tc, tc.tile_pool(name="persist", bufs=1) as persist:

        def _tile(shape, name, dt=f32):
            return persist.tile(shape, dt, name=name)

        # ---- persistent SBUF ------------------------------------------
        wkva_sb = _tile([P, DC, 2 * H], "wkva_sb", f32r)
        wkvb_sb = _tile([P, DC, 2 * H], "wkvb_sb", f32r)
        wqq_sb = _tile([P, DC, 2 * H], "wqq_sb", f32r)
        wrot_sb = _tile([P, DC, 4 * P], "wrot_sb", f16)
        pe0_sb = _tile([P, DC, 512], "pe0_sb", f16)
        bkv_sb = _tile([2 * H, 2], "bkv_sb")
        bqq_sb = _tile([2 * H, 1], "bqq_sb")
        brr_sb = _tile([1, 2 * H], "brr_sb", f16)
        ones16 = _tile([1, 512], "ones16", f16)
        u_cl = _tile([H, 1], "u_cl")
        v_cl = _tile([H, 1], "v_cl")
        u_all = _tile([H, 1], "u_all")
        v_all = _tile([H, 1], "v_all")
        uvf = _tile([2 * H, 2], "uvf")
        uv_colA = _tile([2 * H, 4], "uv_colA", f32r)
        uv_colB = _tile([2 * H, 4], "uv_colB", f32r)
        id_sb = _tile([P, P], "id_sb", f32r)
        warm_sb = _tile([P, 512], "warm_sb", f32r)
        dmask = _tile([P, 4, 512], "dmask", f32r)  # diag-tile 0/1 masks
        K2 = _tile([P, T], "K2", f32r)         # 0:64 r.T+br, 64:128 k.T+bk
        q2T = _tile([P, HALF], "q2T", f32r)    # rows 0:64 and 64:128 = q.T
        valaug = _tile([P, MT, H + 1], "valaug", f32r)
        ebias = _tile([P, MT], "ebias")
        oT_sb = _tile([H + 1, NTQ, 512], "oT_sb")
        outall = _tile([P, HALF // P, H], "outall")

        # ---- DMA queues: big loads on SP ring in critical order, all
        # small transfers on the ACT ring --------------------------------
        nc.sync.dma_start(wkva_sb[:], wkva_r)
        nc.scalar.dma_start(bkv_sb[:], bkv_h[:, :])
        nc.scalar.dma_start(bqq_sb[:], bqq_h[:, :])
        nc.scalar.dma_start(brr_sb[:], brr_h[:, :])
        nc.scalar.dma_start(u_cl[:], u_h[:, :])
        nc.scalar.dma_start(v_cl[:], v_h[:, :])

        with (
            tc.tile_pool(name="xstream", bufs=3) as xpool,
            tc.tile_pool(name="expool", bufs=4) as expool,
            tc.tile_pool(name="pshare", bufs=4, space="PSUM") as pshare,
            tc.tile_pool(name="psc", bufs=2, space="PSUM") as psc,
        ):
            # ---- constants built on device ----------------------------
            with tc.tile_pool(name="setupf", bufs=1) as setupf:
                idf = setupf.tile([P, P], f32, name="idf")
                make_identity(nc, idf[:])
                nc.vector.tensor_copy(id_sb[:], idf[:])
                onesf = setupf.tile([P, MT], f32, name="onesf")
                nc.gpsimd.memset(onesf[:], 1.0)
                nc.vector.tensor_copy(valaug[:, :, H], onesf[:, 0:MT])
                ones_f32 = setupf.tile([1, 512], f32, name="ones_f32")
                nc.gpsimd.memset(ones_f32[:], 1.0)
                nc.vector.tensor_copy(ones16[:], ones_f32[:])
                warmf = setupf.tile([P, 512], f32, name="warmf")
                nc.gpsimd.memset(warmf[:], 0.125)
                nc.vector.tensor_copy(warm_sb[:], warmf[:])
                # 0/1 masks for the four diagonal-tile offsets (keep where
                # key p <= query s, i.e. s - p - 128*rel >= 0)
                dmf = setupf.tile([P, 4, 512], f32, name="dmf")
                nc.gpsimd.memset(dmf[:], 1.0)
                for rel in range(4):
                    nc.gpsimd.affine_select(
                        out=dmf[:, rel, :], in_=dmf[:, rel, :],
                        compare_op=mybir.AluOpType.is_ge, fill=0.0,
                        base=-P * rel, pattern=[[1, 512]],
                        channel_multiplier=-1,
                    )
                nc.vector.tensor_copy(dmask[:], dmf[:])

            # PE pstate warmup: harmless matmuls on scratch data, ready
            # long before the first real operand DMA completes.
            wp = pshare.tile([P, 512], f32, name="wp", tag="sh")
            for _ in range(30):
                nc.tensor.matmul(
                    wp[:], warm_sb[:, 0:P], warm_sb[:], start=True, stop=True
                )

            # key-bias column per chunk half: col 0 (chunks 0-3, rows
            # [r; k]) = [sum(v); sum(u)], col 1 (chunks 4-7, rows [k; r])
            # = [sum(u); sum(v)]
            nc.gpsimd.partition_all_reduce(u_all[:], u_cl[:], H, bass_isa.ReduceOp.add)
            nc.gpsimd.partition_all_reduce(v_all[:], v_cl[:], H, bass_isa.ReduceOp.add)
            nc.vector.tensor_copy(uvf[0:H, 0:1], v_all[:])
            nc.vector.tensor_copy(uvf[0:H, 1:2], u_all[:])
            nc.scalar.dma_start(uvf[H : 2 * H, 0:1], u_all[:])  # partition shift
            nc.scalar.dma_start(uvf[H : 2 * H, 1:2], v_all[:])  # partition shift
            nc.vector.tensor_copy(
                uv_colA[:], uvf[:, 0:1].to_broadcast((2 * H, 4))
            )
            nc.vector.tensor_copy(
                uv_colB[:], uvf[:, 1:2].to_broadcast((2 * H, 4))
            )

            def _r_chunks():
                # r.T chunks, local, no collective.  Pair (g, g+4) in one
                # 128-wide lhsT.  K2's row layout flips per chunk half
                # ([r; k] for chunks 0-3, [k; r] for 4-7 -- valid because
                # q2T is [q; q], so the score contraction is row-order-
                # free), which makes every PSUM drain lane-aligned: no
                # partition-shift DMAs anywhere.
                for g in range(4):
                    rp = pshare.tile([P, 512], f32, name="rp", tag="sh")
                    for dc in range(DC):
                        nc.tensor.matmul(
                            rp[:], wrot_sb[:, dc, g * P : (g + 1) * P],
                            pe0_sb[:, dc, :], start=(dc == 0), stop=False,
                        )
                    nc.tensor.matmul(      # +br on both pair halves (K=1)
                        rp[:], brr_sb[:, :], ones16[:, :],
                        start=False, stop=True,
                    )
                    nc.vector.tensor_copy(
                        K2[0:H, g * 512 : (g + 1) * 512], rp[0:H, :]
                    )
                    nc.vector.tensor_copy(
                        K2[H:P, (g + 4) * 512 : (g + 5) * 512], rp[H:P, :]
                    )

            # ---- streaming sweep: projections + fused attention -------
            ev_state = {}     # (c, tq) -> [av_tile, tiles_done, total]
            tq_seen = set()
            pend = []

            def emit_output(tq):
                for j in range(4):
                    tp = psc.tile([P, 2, 512], f32, name="tp",
                                  tag="sc")[:, 0, 0 : H + 1]
                    nc.tensor.transpose(
                        tp[:], oT_sb[:, tq, j * P : (j + 1) * P],
                        id_sb[0 : H + 1, 0 : H + 1].bitcast(f32),
                    )
                    inv = xpool.tile([P, 1], f32, name="inv", tag="inv",
                                     bufs=2)
                    nc.vector.reciprocal(inv[:], tp[:, H : H + 1])
                    nc.vector.tensor_scalar_mul(
                        outall[:, tq * 4 + j, :], tp[:, 0:H], inv[:]
                    )
                nc.scalar.dma_start(
                    out_r[:, tq * 4 : (tq + 1) * 4, :],
                    outall[:, tq * 4 : (tq + 1) * 4, :],
                )

            def pop_group():
                c, tq, grp, ex2, total = pend.pop(0)
                key = (c, tq)
                st = ev_state.get(key)
                if st is None:
                    av = pshare.tile([P, 512], f32, name="av", tag="sh")
                    st = ev_state[key] = [av, 0, total]
                for i, (mt, rel) in enumerate(grp):
                    nc.tensor.matmul(
                        st[0][0 : H + 1, :], valaug[:, mt, :], ex2[:, i, :],
                        start=(st[1] == 0), stop=(st[1] == total - 1),
                    )
                    st[1] += 1
                if st[1] == total:
                    if tq in tq_seen:
                        nc.vector.tensor_add(
                            oT_sb[:, tq, :], oT_sb[:, tq, :],
                            st[0][0 : H + 1, :],
                        )
                    else:
                        nc.vector.tensor_copy(
                            oT_sb[:, tq, :], st[0][0 : H + 1, :]
                        )
                        tq_seen.add(tq)
                    if c == NCH - 1:
                        emit_output(tq)

            # x-chunk loads: 2-ahead prefetch so the WAR wait on a reused
            # slot is emitted after that slot's readers (emitting all 8
            # upfront serializes the stream against future readers)
            xts = {}

            def _prefetch(c):
                if c >= NCH or c in xts:
                    return
                xt = xpool.tile([P, DC, 512], f32r, name="xt", tag="xt")
                nc.sync.dma_start(xt[:], xT_r[:, :, c * 512 : (c + 1) * 512])
                xts[c] = xt

            _prefetch(0)
            # ring order: everything not needed for kv0 rides behind xt0
            nc.sync.dma_start(wqq_sb[:], wqq_r)
            nc.sync.dma_start(pe0_sb[:], pe0_r)
            nc.sync.dma_start(wrot_sb[:], wrot_r)
            _prefetch(1)
            vstages = {}

            # bracket b: projections of chunk b + attention of chunk b-1.
            # The one-iter shift means every PE instruction's inputs were
            # DVE-drained a full bracket earlier -> no intra-iter stalls.
            for b in range(NCH + 1):
                if b < NCH:
                    c = b
                    if b == 2:
                        # wkvb isn't needed until bracket 4; ride it ahead
                        # of xt4 only
                        nc.sync.dma_start(wkvb_sb[:], wkvb_r)
                    _prefetch(c + 2)
                    sl = slice(c * 512, (c + 1) * 512)
                    lo = c < NTQ      # chunk half: [r; k] rows vs [k; r]
                    wkv_sb = wkva_sb if lo else wkvb_sb
                    var = 0 if lo else 1
                    kvp = pshare.tile([P, 512], f32, name="kvp", tag="sh")
                    for dc in range(DC):
                        nc.tensor.matmul(
                            kvp[:], wkv_sb[:, dc, :], xts[c][:, dc, :],
                            start=(dc == 0), stop=(dc == DC - 1),
                        )
                    # chunks 0-3: kvp rows [v; k], k -> K2 rows 64:128;
                    # chunks 4-7: kvp rows [k; v], k -> K2 rows 0:64.
                    # All drains lane-aligned.
                    vstage = xpool.tile([P, 512], f32r, name="vstage",
                                        tag="vst", bufs=2)
                    vstages[c] = vstage
                    vsl = slice(0, H) if lo else slice(H, P)
                    ksl = slice(H, P) if lo else slice(0, H)
                    nc.vector.tensor_scalar_add(
                        vstage[vsl, :], kvp[vsl, :], bkv_sb[vsl, var : var + 1]
                    )
                    nc.vector.tensor_scalar_add(
                        K2[ksl, sl], kvp[ksl, :], bkv_sb[ksl, var : var + 1]
                    )
                    if c < NTQ:
                        qp = pshare.tile([P, 512], f32, name="qp", tag="sh")
                        for dc in range(DC):
                            nc.tensor.matmul(
                                qp[:], wqq_sb[:, dc, :], xts[c][:, dc, :],
                                start=(dc == 0), stop=(dc == DC - 1),
                            )
                        nc.vector.tensor_scalar_add(
                            q2T[:, sl], qp[:], bqq_sb[:]
                        )

                if b == 0:
                    # bridge the wrot-DMA wait with more warmup, then
                    # compute r (needed first by bracket 1's bias matmuls)
                    wp2 = pshare.tile([P, 512], f32, name="wp2", tag="sh")
                    for _ in range(18):
                        nc.tensor.matmul(
                            wp2[:], warm_sb[:, 0:P], warm_sb[:],
                            start=True, stop=True,
                        )
                    _r_chunks()
                    continue
                c = b - 1
                # v transposes + key-bias matmuls share one PSUM tile
                # (disjoint column regions)
                vb = pshare.tile([P, 512], f32, name="vb", tag="sh")
                vstage = vstages.pop(c)
                cvsl = slice(0, H) if c < NTQ else slice(H, P)
                cvar = 0 if c < NTQ else 1
                for j in range(4):
                    nc.tensor.transpose(
                        vb[:, j * H : (j + 1) * H].bitcast(f32r),
                        vstage[cvsl, j * P : (j + 1) * P],
                        id_sb[cvsl, cvsl],
                    )
                uv_col = uv_colA if cvar == 0 else uv_colB
                for j in range(4):
                    mt = c * 4 + j
                    msl = slice(mt * P, (mt + 1) * P)
                    nc.tensor.matmul(
                        vb[:, 4 * H + 4 * j : 4 * H + 4 * j + 4], K2[:, msl],
                        uv_col[:], start=True, stop=True,
                    )
                nc.scalar.activation(
                    ebias[:, c * 4 : (c + 1) * 4],
                    vb[:, 4 * H : 4 * H + 16 : 4], Exp,
                )
                for j in range(4):
                    mt = c * 4 + j
                    nc.vector.tensor_scalar_mul(
                        valaug[:, mt, 0:H],
                        vb[:, j * H : (j + 1) * H].bitcast(f32r),
                        ebias[:, mt : mt + 1],
                    )
                    nc.vector.tensor_copy(
                        valaug[:, mt, H : H + 1], ebias[:, mt : mt + 1]
                    )

                # fused attention for everything unlocked by chunk c
                for tq, tiles in _attn_events(c):
                    tsl = slice(tq * 512, (tq + 1) * 512)
                    total = len(tiles)
                    for gi in range(0, total, 2):
                        grp = tiles[gi : gi + 2]
                        sp2 = psc.tile([P, 2, 512], f32, name="sp", tag="sc")
                        for i, (mt, rel) in enumerate(grp):
                            msl = slice(mt * P, (mt + 1) * P)
                            nc.tensor.matmul(
                                sp2[:, i, :], K2[:, msl], q2T[:, tsl],
                                start=True, stop=True,
                            )
                        ex2 = expool.tile([P, 2, 512], f32r, name="ex",
                                          tag="ex")
                        nc.scalar.activation(ex2[:], sp2[:], Exp)
                        for i, (mt, rel) in enumerate(grp):
                            if rel is not None:
                                # zero masked entries on the idle Pool
                                # engine (f32r ALU output is rounded, so
                                # the attnval matmul accepts it)
                                nc.gpsimd.tensor_mul(
                                    ex2[:, i, :], ex2[:, i, :],
                                    dmask[:, rel, :],
                                )
                        pend.append((c, tq, grp, ex2, total))
                        if len(pend) > LAGG:
                            pop_group()
            while pend:
                pop_group()

    nc.compile()
    return nc


def _get_module():
    if "nc" not in _CACHE:
        _CACHE["nc"] = _build_module()
    return _CACHE["nc"]


def _make_in_maps(inputs):
    inp = np.asarray(inputs["inp_data"], dtype=np.float32)
    Wq = np.asarray(inputs["Wq"], dtype=np.float32)
    bq = np.asarray(inputs["bq"], dtype=np.float32)
    Wk = np.asarray(inputs["Wk"], dtype=np.float32)
    bk = np.asarray(inputs["bk"], dtype=np.float32)
    Wv = np.asarray(inputs["Wv"], dtype=np.float32)
    bv = np.asarray(inputs["bv"], dtype=np.float32)
    Wr = np.asarray(inputs["Wr"], dtype=np.float32)
    br = np.asarray(inputs["br"], dtype=np.float32)
    u = np.asarray(inputs["u"], dtype=np.float32)
    v = np.asarray(inputs["v"], dtype=np.float32)

    if "pe0" not in _CACHE:
        _CACHE["pe0"] = np.ascontiguousarray(_pe0T().astype(np.float16))
    common = {
        "pe0": _CACHE["pe0"],
        "wrot": np.ascontiguousarray(_rotate_wr(Wr).astype(np.float16)),
        "wkva": np.ascontiguousarray(np.concatenate([Wv, Wk], axis=1)),
        "wkvb": np.ascontiguousarray(np.concatenate([Wk, Wv], axis=1)),
        "wqq": np.ascontiguousarray(np.concatenate([Wq, Wq], axis=1)),
        "bkv": np.ascontiguousarray(
            np.stack(
                [np.concatenate([bv, bk]), np.concatenate([bk, bv])], axis=1
            )
        ),
        "bqq": np.ascontiguousarray(np.concatenate([bq, bq]).reshape(2 * H, 1)),
        "brr": np.ascontiguousarray(
            np.concatenate([br, br]).reshape(1, 2 * H).astype(np.float16)
        ),
        "u": np.ascontiguousarray(u.reshape(H, 1)),
        "v": np.ascontiguousarray(v.reshape(H, 1)),
    }
    in_maps = []
    for b in range(NCORES):
        m = {"xT": np.ascontiguousarray(inp[:, b, :].T)}
        m.update(common)
        in_maps.append(m)
    return in_maps


def _run(in_maps, trace=False):
    from concourse.bass_utils import run_bass_kernel_spmd

    nc = _get_module()
    return run_bass_kernel_spmd(
        nc, in_maps, core_ids=list(range(NCORES)), trace=trace
    )


def _timed_run(in_maps, iters=5, reps=1):
    """Replicates bass2jax.run_bass_via_pjrt's multi-core path, but keeps the
    jitted callable and device-resident inputs so repeated executions can be
    wall-clock timed (no NTFF profiling is available through the axon client).
    """
    import time

    import jax
    import concourse.mybir as mybir
    from concourse.bass2jax import (
        _bass_exec_p,
        install_neuronx_cc_hook,
        partition_id_tensor,
    )
    from jax.experimental.shard_map import shard_map
    from jax.sharding import Mesh, NamedSharding, PartitionSpec

    nc = _get_module()
    install_neuronx_cc_hook()
    partition_name = nc.partition_id_tensor.name if nc.partition_id_tensor else None

    in_names, out_names, out_avals, zero_shapes = [], [], [], []
    for alloc in nc.m.functions[0].allocations:
        if not isinstance(alloc, mybir.MemoryLocationSet):
            continue
        name = alloc.memorylocations[0].name
        if alloc.kind == "ExternalInput":
            if name != partition_name:
                in_names.append(name)
        elif alloc.kind == "ExternalOutput":
            out_names.append(name)
            shape = tuple(alloc.tensor_shape)
            dtype = mybir.dt.np(alloc.dtype)
            out_avals.append(jax.core.ShapedArray(shape, dtype))
            zero_shapes.append((shape, dtype))
    n_params = len(in_names)
    all_names = in_names + out_names
    if partition_name is not None:
        all_names = all_names + [partition_name]
    donate = tuple(range(n_params, n_params + len(out_names)))

    def _body(*args):
        operands = list(args)
        if partition_name is not None:
            operands.append(partition_id_tensor())
        outs = _bass_exec_p.bind(
            *operands,
            out_avals=tuple(out_avals),
            in_names=tuple(all_names),
            out_names=tuple(out_names),
            lowering_input_output_aliases=(),
            sim_require_finite=True,
            sim_require_nnan=True,
            nc=nc,
        )
        return tuple(outs)

    devices = jax.devices()[:NCORES]
    mesh = Mesh(np.asarray(devices), ("core",))
    spec = PartitionSpec("core")
    in_specs = (spec,) * (n_params + len(out_names))
    sharded = jax.jit(
        shard_map(
            _body, mesh=mesh, in_specs=in_specs,
            out_specs=(spec,) * len(out_names), check_rep=False,
        ),
        donate_argnums=donate,
        keep_unused=True,
    )
    sharding = NamedSharding(mesh, spec)
    concat_in = [
        jax.device_put(
            np.concatenate([in_maps[c][nm] for c in range(NCORES)], axis=0), sharding
        )
        for nm in in_names
    ]

    def zeros():
        return [
            jax.device_put(np.zeros((NCORES * s[0], *s[1:]), d), sharding)
            for (s, d) in zero_shapes
        ]

    out = sharded(*concat_in, *zeros())
    jax.block_until_ready(out)
    times = []
    for _ in range(iters):
        zs = zeros()
        jax.block_until_ready(zs)
        t0 = time.perf_counter()
        out = sharded(*concat_in, *zs)
        jax.block_until_ready(out)
        times.append(time.perf_counter() - t0)
    results = {
        nm: np.asarray(out[i]).reshape(NCORES, *out_avals[i].shape)
        for i, nm in enumerate(out_names)
    }
    return results, times


def kernel(**inputs) -> np.ndarray:
    in_maps = _make_in_maps(inputs)
    res = _run(in_maps, trace=False)
    out = np.stack([res.results[b]["out"] for b in range(NCORES)], axis=1)
    return np.ascontiguousarray(out.astype(np.float32))


# revision 60
# speedup vs baseline: 1.3629x; 1.0749x over previous
"""Trainium2 Bass kernel for nn_Attention_Module_15152644620833 (v5).

Reference computation (T=4096, B=8, D=1024, H=64, half=2048):
    q   = x[:half] @ Wq + bq            (half, B, H)
    k   = x @ Wk + bk                   (T, B, H)
    val = x @ Wv + bv                   (T, B, H)
    r   = posenc(T, D) @ Wr + br        (T, H)
    scores[b] = q[b] @ (k[b] + r).T + bias[b][None, :]
        where bias[b][m] = sum(u) * k[m,b,:].sum() + sum(v) * r[m,:].sum()
    causal mask on first `half` key positions, softmax over all T keys,
    out = attn @ val                    (half, B, H)

Sharding: data-parallel over batch, one batch element per NeuronCore.

v5 design (single-sweep pipeline, no collective):
  * r is computed locally on every core with zero communication.  Because
    pe[t] is sinusoidal, r.T for key chunk c equals W'_c.T @ pe0.T where
    pe0 is the FIRST 512-row block of the positional encoding and W'_c is
    Wr with its (sin, cos) row pairs rotated by the chunk phase
    phi_i = 512*c*div_i (host-side weight prep, exact identity).  Chunk
    pairs are packed into one 128-wide lhsT so the 8 chunks cost only
    4x8 matmuls; +br rides a K=1 matmul with a ones row.  pe0/W' are fp16
    (|pe|<=1; rel err ~6e-4 -> negligible vs the 2e-2 gate).
  * K2 (128, T) holds r.T on rows 0:64 (pair row 0:64 drains via DVE, the
    odd-chunk half via a partition-shift DMA) and k.T on rows 64:128;
    q2T holds [q; q] so scoresT = K2[:,msl].T @ q2T in one K=128 matmul.
  * key bias sum(u)*k_sum + sum(v)*r_sum folds multiplicatively into the
    val tiles (exp(s+b) = exp(s)*eb; eb scales both the val columns and
    the denominator ones column, so attn is unchanged, exactly).
  * causal mask: exp runs unmasked (|s|<~70 is fp32-safe); masked entries
    of diagonal-tile exp results are zeroed by a Pool affine_select, so
    the PE never runs mask matmuls.
  * attention is fused into the k/v streaming loop: when x chunk c lands,
    its projections run and every (query-chunk, key-tile) pair that just
    became computable is scored immediately.  Scores run in 2-tile groups
    so each activation-engine exp covers 1024 columns (the per-op PSUM
    access penalty is ~143ns, so per-tile exps would make ACT the
    bottleneck).  attnval accumulates per (iter, tq) event in a PSUM bank,
    then DVE-accumulates into an SBUF oT accumulator.  Score -> exp ->
    attnval is software-pipelined 2 groups deep so the in-order PE queue
    never waits on the activation engine.
  * DMA: the SP ring carries only the big loads in critical order
    (wkv, wqq, pe0, wrot, x chunks); all small transfers ride the ACT
    ring (each DMA costs ~650ns of ring-queue time regardless of size).
  * a dozen warmup matmuls on scratch data ramp the PE DVFS pstate to
    full clock before the first real matmul arrives.
"""

import math

import numpy as np

T, B, D, H = 4096, 8, 1024, 64
HALF = T // 2
P = 128
DC = D // P          # 8 d-chunks
NCH = T // 512       # 8 key/x chunks of 512
NTQ = HALF // 512    # 4 query chunks of 512
MT = T // P          # 32 key tiles of 128
NCORES = 8
LAGG = 3             # software-pipeline lag, in 2-tile score groups

_CACHE = {}


def _div_vec():
    return np.exp(
        np.arange(0, D, 2, dtype=np.float64) * (-(math.log(10000.0) / D))
    )


def _pe0T():
    """First 512 rows of the sin/cos positional encoding, transposed
    to (D, 512)."""
    div = _div_vec()
    ang = np.arange(512, dtype=np.float64)[:, None] * div[None, :]
    pe0 = np.stack([np.sin(ang), np.cos(ang)], axis=-1).reshape(512, D)
    return np.ascontiguousarray(pe0.T)


def _rotate_wr(Wr):
    """W'_c for every chunk c, packed in pairs (g, g+4): (D, 512) with
    columns [g*128 : g*128+64] = W'_g and [g*128+64 : (g+1)*128] =
    W'_{g+4}.  The early chunk of each pair lands on PSUM rows 0:64
    (lane-aligned DVE drain, available immediately); the late chunk's
    partition-shift DMA can wait for a crack in the x-stream.

    Identity: r[512c+s, h] = sum_d pe0[s, d] * W'_c[d, h]  (+ br), where
    W'_c rotates each (sin,cos) row pair of Wr by phi_i = 512*c*div_i.
    """
    div = _div_vec()
    Wr = np.asarray(Wr, dtype=np.float64)
    We, Wo = Wr[0::2, :], Wr[1::2, :]

    def _wc(c):
        phi = 512.0 * c * div
        cph, sph = np.cos(phi)[:, None], np.sin(phi)[:, None]
        Wc = np.empty((D, H), dtype=np.float64)
        Wc[0::2, :] = cph * We - sph * Wo
        Wc[1::2, :] = sph * We + cph * Wo
        return Wc

    cols = []
    for g in range(4):
        cols += [_wc(g), _wc(g + 4)]
    return np.ascontiguousarray(np.concatenate(cols, axis=1))


def _attn_events(c):
    """Events for iter c: list of (tq, [(mt, rel), ...]).  rel is the
    diagonal-tile offset (None for fully-unmasked tiles)."""
    if c < NTQ:
        tiles = []
        for kc in range(c):
            tiles += [(4 * kc + j, None) for j in range(4)]
        tiles += [(4 * c + j, j) for j in range(4)]
        return [(c, tiles)]
    return [(tq, [(4 * c + j, None) for j in range(4)]) for tq in range(NTQ)]


def _build_module():
    import concourse.bacc as bacc
    import concourse.bass_isa as bass_isa
    import concourse.mybir as mybir
    from concourse.masks import make_identity
    from concourse.tile import TileContext

    f32 = mybir.dt.float32
    f32r = mybir.dt.float32r
    f16 = mybir.dt.float16
    Exp = mybir.ActivationFunctionType.Exp

    nc = bacc.Bacc(num_devices=NCORES)

    xT_h = nc.dram_tensor("xT", [D, T], f32r, kind="ExternalInput")
    pe0_h = nc.dram_tensor("pe0", [D, 512], f16, kind="ExternalInput")
    wrot_h = nc.dram_tensor("wrot", [D, 512], f16, kind="ExternalInput")
    wkva_h = nc.dram_tensor("wkva", [D, 2 * H], f32r, kind="ExternalInput")
    wkvb_h = nc.dram_tensor("wkvb", [D, 2 * H], f32r, kind="ExternalInput")
    wqq_h = nc.dram_tensor("wqq", [D, 2 * H], f32r, kind="ExternalInput")
    # one combined small-constant tensor (a single early DMA; tiny DMAs
    # on a saturated engine otherwise wait ~30us for a crack in the
    # x-stream): cols [bkvA 0:1 | bkvB 1:2 | bqq 2:3 | uvcA 3:7 |
    # uvcB 7:11 | br2 11:12]
    sm_h = nc.dram_tensor("smalls", [2 * H, 12], f32, kind="ExternalInput")
    out_h = nc.dram_tensor("out", [HALF, H], f32, kind="ExternalOutput")

    xT_r = xT_h[:, :].rearrange("(c p) t -> p c t", p=P)       # (128, 8, T)
    pe0_r = pe0_h[:, :].rearrange("(c p) s -> p c s", p=P)     # (128, 8, 512)
    wrot_r = wrot_h[:, :].rearrange("(c p) h -> p c h", p=P)
    wkva_r = wkva_h[:, :].rearrange("(c p) h -> p c h", p=P)
    wkvb_r = wkvb_h[:, :].rearrange("(c p) h -> p c h", p=P)
    wqq_r = wqq_h[:, :].rearrange("(c p) h -> p c h", p=P)
    out_r = out_h[:, :].rearrange("(g p) h -> p g h", p=P)     # (128, 16, 64)

    with TileContext(nc) as tc, tc.tile_pool(name="persist", bufs=1) as persist:

        def _tile(shape, name, dt=f32):
            return persist.tile(shape, dt, name=name)

        # ---- persistent SBUF ------------------------------------------
        wkva_sb = _tile([P, DC, 2 * H], "wkva_sb", f32r)
        wkvb_sb = _tile([P, DC, 2 * H], "wkvb_sb", f32r)
        wqq_sb = _tile([P, DC, 2 * H], "wqq_sb", f32r)
        wrot_sb = _tile([P, DC, 4 * P], "wrot_sb", f16)
        pe0_sb = _tile([P, DC, 512], "pe0_sb", f16)
        sm_sb = _tile([2 * H, 12], "sm_sb")
        uv_colA = _tile([2 * H, 4], "uv_colA", f32r)
        uv_colB = _tile([2 * H, 4], "uv_colB", f32r)
        id_sb = _tile([P, P], "id_sb", f32r)
        warm_sb = _tile([P, 512], "warm_sb", f32r)
        dmask = _tile([P, 4, 512], "dmask", f32r)  # diag-tile 0/1 masks
        K2 = _tile([P, T], "K2", f32r)         # 0:64 r.T+br, 64:128 k.T+bk
        q2T = _tile([P, HALF], "q2T", f32r)    # rows 0:64 and 64:128 = q.T
        valaug = _tile([P, MT, H + 1], "valaug", f32r)
        ebias = _tile([P, MT], "ebias")
        oT_sb = _tile([H + 1, NTQ, 512], "oT_sb")
        outall = _tile([P, HALF // P, H], "outall")

        # ---- DMA queue: SP ring in critical order ----------------------
        nc.sync.dma_start(sm_sb[:], sm_h[:, :])
        nc.sync.dma_start(wkva_sb[:], wkva_r)

        with (
            tc.tile_pool(name="xstream", bufs=3) as xpool,
            tc.tile_pool(name="expool", bufs=5) as expool,
            tc.tile_pool(name="pshare", bufs=4, space="PSUM") as pshare,
            tc.tile_pool(name="psc", bufs=2, space="PSUM") as psc,
        ):
            # ---- constants built on device ----------------------------
            with tc.tile_pool(name="setupf", bufs=1) as setupf:
                warmf = setupf.tile([P, 512], f32, name="warmf")
                nc.gpsimd.memset(warmf[:], 0.125)
                nc.vector.tensor_copy(warm_sb[:], warmf[:])
                idf = setupf.tile([P, P], f32, name="idf")
                make_identity(nc, idf[:])
                nc.vector.tensor_copy(id_sb[:], idf[:])
                onesf = setupf.tile([P, MT], f32, name="onesf")
                nc.gpsimd.memset(onesf[:], 1.0)
                nc.vector.tensor_copy(valaug[:, :, H], onesf[:, 0:MT])
                # 0/1 masks for the four diagonal-tile offsets (keep where
                # key p <= query s, i.e. s - p - 128*rel >= 0)
                dmf = setupf.tile([P, 4, 512], f32, name="dmf")
                nc.gpsimd.memset(dmf[:], 1.0)
                for rel in range(4):
                    nc.gpsimd.affine_select(
                        out=dmf[:, rel, :], in_=dmf[:, rel, :],
                        compare_op=mybir.AluOpType.is_ge, fill=0.0,
                        base=-P * rel, pattern=[[1, 512]],
                        channel_multiplier=-1,
                    )
                nc.vector.tensor_copy(dmask[:], dmf[:])

            # PE pstate warmup: harmless matmuls on scratch data, ready
            # long before the first real operand DMA completes.
            wp = pshare.tile([P, 512], f32, name="wp", tag="sh")
            for _ in range(29):
                nc.tensor.matmul(
                    wp[:], warm_sb[:, 0:P], warm_sb[:], start=True, stop=True
                )

            # key-bias columns (sum(u)/sum(v) are host-computed scalars in
            # the smalls tensor): chunks 0-3 rows [r; k] -> [sum(v);
            # sum(u)], chunks 4-7 rows [k; r] -> [sum(u); sum(v)]
            nc.vector.tensor_copy(uv_colA[:], sm_sb[:, 3:7])
            nc.vector.tensor_copy(uv_colB[:], sm_sb[:, 7:11])

            def _r_chunks():
                # r.T chunks, local, no collective.  Pair (g, g+4) in one
                # 128-wide lhsT.  K2's row layout flips per chunk half
                # ([r; k] for chunks 0-3, [k; r] for 4-7 -- valid because
                # q2T is [q; q], so the score contraction is row-order-
                # free), which makes every PSUM drain lane-aligned: no
                # partition-shift DMAs anywhere.
                for g in range(4):
                    rp = pshare.tile([P, 512], f32, name="rp", tag="sh")
                    for dc in range(DC):
                        nc.tensor.matmul(
                            rp[:], wrot_sb[:, dc, g * P : (g + 1) * P],
                            pe0_sb[:, dc, :],
                            start=(dc == 0), stop=(dc == DC - 1),
                        )
                    nc.vector.tensor_scalar_add(
                        K2[0:H, g * 512 : (g + 1) * 512], rp[0:H, :],
                        sm_sb[0:H, 11:12],
                    )
                    nc.vector.tensor_scalar_add(
                        K2[H:P, (g + 4) * 512 : (g + 5) * 512], rp[H:P, :],
                        sm_sb[H : 2 * H, 11:12],
                    )

            # ---- streaming sweep: projections + fused attention -------
            ev_state = {}     # (c, tq) -> [av_tile, tiles_done, total]
            tq_seen = set()
            pend = []

            def emit_output(tq):
                for j in range(4):
                    tp = psc.tile([P, 2, 512], f32, name="tp",
                                  tag="sc")[:, 0, 0 : H + 1] if False else \
                        pshare.tile([P, 512], f32, name="tp",
                                    tag="sh")[:, 0 : H + 1]
                    nc.tensor.transpose(
                        tp[:], oT_sb[:, tq, j * P : (j + 1) * P],
                        id_sb[0 : H + 1, 0 : H + 1].bitcast(f32),
                    )
                    inv = xpool.tile([P, 1], f32, name="inv", tag="inv",
                                     bufs=2)
                    nc.vector.reciprocal(inv[:], tp[:, H : H + 1])
                    nc.vector.tensor_scalar_mul(
                        outall[:, tq * 4 + j, :], tp[:, 0:H], inv[:]
                    )
                nc.scalar.dma_start(
                    out_r[:, tq * 4 : (tq + 1) * 4, :],
                    outall[:, tq * 4 : (tq + 1) * 4, :],
                )

            def pop_group():
                c, tq, grp, ex2, total = pend.pop(0)
                key = (c, tq)
                st = ev_state.get(key)
                if st is None:
                    av = pshare.tile([P, 512], f32, name="av", tag="sh")
                    st = ev_state[key] = [av, 0, total]
                for i, (mt, rel) in enumerate(grp):
                    nc.tensor.matmul(
                        st[0][0 : H + 1, :], valaug[:, mt, :], ex2[:, i, :],
                        start=(st[1] == 0), stop=(st[1] == total - 1),
                    )
                    st[1] += 1
                if st[1] == total:
                    if tq in tq_seen:
                        nc.vector.tensor_add(
                            oT_sb[:, tq, :], oT_sb[:, tq, :],
                            st[0][0 : H + 1, :],
                        )
                    else:
                        nc.vector.tensor_copy(
                            oT_sb[:, tq, :], st[0][0 : H + 1, :]
                        )
                        tq_seen.add(tq)
                    # delay each output stage one event so its transposes
                    # never wait on the just-issued DVE accumulation
                    if c == NCH - 1 and tq > 0:
                        emit_output(tq - 1)

            # x-chunk loads: 2-ahead prefetch so the WAR wait on a reused
            # slot is emitted after that slot's readers (emitting all 8
            # upfront serializes the stream against future readers)
            xts = {}

            def _prefetch(c):
                if c >= NCH or c in xts:
                    return
                xt = xpool.tile([P, DC, 512], f32r, name="xt", tag="xt")
                nc.sync.dma_start(xt[:], xT_r[:, :, c * 512 : (c + 1) * 512])
                xts[c] = xt

            _prefetch(0)
            # ring order: everything not needed for kv0 rides behind xt0;
            # pe0/wrot (for r) ahead of wqq (q0 is emitted after r)
            nc.sync.dma_start(pe0_sb[:], pe0_r)
            nc.sync.dma_start(wrot_sb[:], wrot_r)
            nc.sync.dma_start(wqq_sb[:], wqq_r)
            _prefetch(1)
            vstages = {}

            # bracket b: projections of chunk b + attention of chunk b-1.
            # The one-iter shift means every PE instruction's inputs were
            # DVE-drained a full bracket earlier -> no intra-iter stalls.
            for b in range(NCH + 1):
                if b < NCH:
                    c = b
                    if b == 2:
                        # wkvb isn't needed until bracket 4; ride it ahead
                        # of xt4 only
                        nc.sync.dma_start(wkvb_sb[:], wkvb_r)
                    _prefetch(c + 2)
                    sl = slice(c * 512, (c + 1) * 512)
                    lo = c < NTQ      # chunk half: [r; k] rows vs [k; r]
                    wkv_sb = wkva_sb if lo else wkvb_sb
                    var = 0 if lo else 1
                    kvp = pshare.tile([P, 512], f32, name="kvp", tag="sh")
                    for dc in range(DC):
                        nc.tensor.matmul(
                            kvp[:], wkv_sb[:, dc, :], xts[c][:, dc, :],
                            start=(dc == 0), stop=(dc == DC - 1),
                        )
                    # chunks 0-3: kvp rows [v; k], k -> K2 rows 64:128;
                    # chunks 4-7: kvp rows [k; v], k -> K2 rows 0:64.
                    # All drains lane-aligned.
                    vstage = xpool.tile([P, 512], f32r, name="vstage",
                                        tag="vst", bufs=2)
                    vstages[c] = vstage
                    vsl = slice(0, H) if lo else slice(H, P)
                    ksl = slice(H, P) if lo else slice(0, H)
                    nc.vector.tensor_scalar_add(
                        vstage[vsl, :], kvp[vsl, :], sm_sb[vsl, var : var + 1]
                    )
                    nc.vector.tensor_scalar_add(
                        K2[ksl, sl], kvp[ksl, :], sm_sb[ksl, var : var + 1]
                    )
                    def _q_proj(c, sl):
                        qp = pshare.tile([P, 512], f32, name="qp", tag="sh")
                        for dc in range(DC):
                            nc.tensor.matmul(
                                qp[:], wqq_sb[:, dc, :], xts[c][:, dc, :],
                                start=(dc == 0), stop=(dc == DC - 1),
                            )
                        nc.vector.tensor_scalar_add(
                            q2T[:, sl], qp[:], sm_sb[:, 2:3]
                        )

                    if c < NTQ and b != 0:
                        _q_proj(c, sl)

                if b == 0:
                    # bridge the wrot-DMA wait with more warmup, then
                    # compute r (needed first by bracket 1's bias matmuls);
                    # q0 last (wqq rides behind wrot on the ring)
                    wp2 = pshare.tile([P, 512], f32, name="wp2", tag="sh")
                    for _ in range(5):
                        nc.tensor.matmul(
                            wp2[:], warm_sb[:, 0:P], warm_sb[:],
                            start=True, stop=True,
                        )
                    _r_chunks()
                    _q_proj(0, slice(0, 512))
                    continue
                c = b - 1
                # v transposes + key-bias matmuls share one PSUM tile
                # (disjoint column regions)
                vb = pshare.tile([P, 512], f32, name="vb", tag="sh")
                vstage = vstages.pop(c)
                cvsl = slice(0, H) if c < NTQ else slice(H, P)
                cvar = 0 if c < NTQ else 1
                for j in range(4):
                    nc.tensor.transpose(
                        vb[:, j * H : (j + 1) * H].bitcast(f32r),
                        vstage[cvsl, j * P : (j + 1) * P],
                        id_sb[cvsl, cvsl],
                    )
                uv_col = uv_colA if cvar == 0 else uv_colB
                for j in range(4):
                    mt = c * 4 + j
                    msl = slice(mt * P, (mt + 1) * P)
                    nc.tensor.matmul(
                        vb[:, 4 * H + 4 * j : 4 * H + 4 * j + 4], K2[:, msl],
                        uv_col[:], start=True, stop=True,
                    )
                nc.scalar.activation(
                    ebias[:, c * 4 : (c + 1) * 4],
                    vb[:, 4 * H : 4 * H + 16 : 4], Exp,
                )
                for j in range(4):
                    mt = c * 4 + j
                    nc.vector.tensor_scalar_mul(
                        valaug[:, mt, 0:H],
                        vb[:, j * H : (j + 1) * H].bitcast(f32r),
                        ebias[:, mt : mt + 1],
                    )
                    nc.vector.tensor_copy(
                        valaug[:, mt, H : H + 1], ebias[:, mt : mt + 1]
                    )

                # fused attention for everything unlocked by chunk c
                for tq, tiles in _attn_events(c):
                    tsl = slice(tq * 512, (tq + 1) * 512)
                    total = len(tiles)
                    for gi in range(0, total, 2):
                        grp = tiles[gi : gi + 2]
                        sp2 = psc.tile([P, 2, 512], f32, name="sp", tag="sc")
                        for i, (mt, rel) in enumerate(grp):
                            msl = slice(mt * P, (mt + 1) * P)
                            nc.tensor.matmul(
                                sp2[:, i, :], K2[:, msl], q2T[:, tsl],
                                start=True, stop=True,
                            )
                        ex2 = expool.tile([P, 2, 512], f32r, name="ex",
                                          tag="ex")
                        nc.scalar.activation(ex2[:], sp2[:], Exp)
                        for i, (mt, rel) in enumerate(grp):
                            if rel is not None:
                                # zero masked entries on the idle Pool
                                # engine (f32r ALU output is rounded, so
                                # the attnval matmul accepts it)
                                nc.gpsimd.tensor_mul(
                                    ex2[:, i, :], ex2[:, i, :],
                                    dmask[:, rel, :],
                                )
                        pend.append((c, tq, grp, ex2, total))
                        if len(pend) > LAGG:
                            pop_group()
            while pend:
                pop_group()

    nc.compile()
    return nc


def _get_module():
    if "nc" not in _CACHE:
        _CACHE["nc"] = _build_module()
    return _CACHE["nc"]


def _make_in_maps(inputs):
    inp = np.asarray(inputs["inp_data"], dtype=np.float32)
    Wq = np.asarray(inputs["Wq"], dtype=np.float32)
    bq = np.asarray(inputs["bq"], dtype=np.float32)
    Wk = np.asarray(inputs["Wk"], dtype=np.float32)
    bk = np.asarray(inputs["bk"], dtype=np.float32)
    Wv = np.asarray(inputs["Wv"], dtype=np.float32)
    bv = np.asarray(inputs["bv"], dtype=np.float32)
    Wr = np.asarray(inputs["Wr"], dtype=np.float32)
    br = np.asarray(inputs["br"], dtype=np.float32)
    u = np.asarray(inputs["u"], dtype=np.float32)
    v = np.asarray(inputs["v"], dtype=np.float32)

    if "pe0" not in _CACHE:
        _CACHE["pe0"] = np.ascontiguousarray(_pe0T().astype(np.float16))
    us, vs = np.float32(u.sum()), np.float32(v.sum())
    sm = np.zeros((2 * H, 12), dtype=np.float32)
    sm[:, 0] = np.concatenate([bv, bk])       # bkv, chunks 0-3 ([v; k])
    sm[:, 1] = np.concatenate([bk, bv])       # bkv, chunks 4-7 ([k; v])
    sm[:, 2] = np.concatenate([bq, bq])       # bqq
    sm[0:H, 3:7], sm[H:, 3:7] = vs, us        # uv col, chunks 0-3 [r; k]
    sm[0:H, 7:11], sm[H:, 7:11] = us, vs      # uv col, chunks 4-7 [k; r]
    sm[0:H, 11] = br                          # br on both pair halves
    sm[H:, 11] = br
    common = {
        "pe0": _CACHE["pe0"],
        "wrot": np.ascontiguousarray(_rotate_wr(Wr).astype(np.float16)),
        "wkva": np.ascontiguousarray(np.concatenate([Wv, Wk], axis=1)),
        "wkvb": np.ascontiguousarray(np.concatenate([Wk, Wv], axis=1)),
        "wqq": np.ascontiguousarray(np.concatenate([Wq, Wq], axis=1)),
        "smalls": sm,
    }
    in_maps = []
    for b in range(NCORES):
        m = {"xT": np.ascontiguousarray(inp[:, b, :].T)}
        m.update(common)
        in_maps.append(m)
    return in_maps


def _run(in_maps, trace=False):
    from concourse.bass_utils import run_bass_kernel_spmd

    nc = _get_module()
    return run_bass_kernel_spmd(
        nc, in_maps, core_ids=list(range(NCORES)), trace=trace
    )


def _timed_run(in_maps, iters=5, reps=1):
    """Replicates bass2jax.run_bass_via_pjrt's multi-core path, but keeps the
    jitted callable and device-resident inputs so repeated executions can be
    wall-clock timed (no NTFF profiling is available through the axon client).
    """
    import time

    import jax
    import concourse.mybir as mybir
    from concourse.bass2jax import (
        _bass_exec_p,
        install_neuronx_cc_hook,
        partition_id_tensor,
    )
    from jax.experimental.shard_map import shard_map
    from jax.sharding import Mesh, NamedSharding, PartitionSpec

    nc = _get_module()
    install_neuronx_cc_hook()
    partition_name = nc.partition_id_tensor.name if nc.partition_id_tensor else None

    in_names, out_names, out_avals, zero_shapes = [], [], [], []
    for alloc in nc.m.functions[0].allocations:
        if not isinstance(alloc, mybir.MemoryLocationSet):
            continue
        name = alloc.memorylocations[0].name
        if alloc.kind == "ExternalInput":
            if name != partition_name:
                in_names.append(name)
        elif alloc.kind == "ExternalOutput":
            out_names.append(name)
            shape = tuple(alloc.tensor_shape)
            dtype = mybir.dt.np(alloc.dtype)
            out_avals.append(jax.core.ShapedArray(shape, dtype))
            zero_shapes.append((shape, dtype))
    n_params = len(in_names)
    all_names = in_names + out_names
    if partition_name is not None:
        all_names = all_names + [partition_name]
    donate = tuple(range(n_params, n_params + len(out_names)))

    def _body(*args):
        operands = list(args)
        if partition_name is not None:
            operands.append(partition_id_tensor())
        outs = _bass_exec_p.bind(
            *operands,
            out_avals=tuple(out_avals),
            in_names=tuple(all_names),
            out_names=tuple(out_names),
            lowering_input_output_aliases=(),
            sim_require_finite=True,
            sim_require_nnan=True,
            nc=nc,
        )
        return tuple(outs)

    devices = jax.devices()[:NCORES]
    mesh = Mesh(np.asarray(devices), ("core",))
    spec = PartitionSpec("core")
    in_specs = (spec,) * (n_params + len(out_names))
    sharded = jax.jit(
        shard_map(
            _body, mesh=mesh, in_specs=in_specs,
            out_specs=(spec,) * len(out_names), check_rep=False,
        ),
        donate_argnums=donate,
        keep_unused=True,
    )
    sharding = NamedSharding(mesh, spec)
    concat_in = [
        jax.device_put(
            np.concatenate([in_maps[c][nm] for c in range(NCORES)], axis=0), sharding
        )
        for nm in in_names
    ]

    def zeros():
        return [
            jax.device_put(np.zeros((NCORES * s[0], *s[1:]), d), sharding)
            for (s, d) in zero_shapes
        ]

    out = sharded(*concat_in, *zeros())
    jax.block_until_ready(out)
    times = []
    for _ in range(iters):
        zs = zeros()
        jax.block_until_ready(zs)
        t0 = time.perf_counter()
        out = sharded(*concat_in, *zs)
        jax.block_until_ready(out)
        times.append(time.perf_counter() - t0)
    results = {
        nm: np.asarray(out[i]).reshape(NCORES, *out_avals[i].shape)
        for i, nm in enumerate(out_names)
    }
    return results, times


def kernel(**inputs) -> np.ndarray:
    in_maps = _make_in_maps(inputs)
    res = _run(in_maps, trace=False)
    out = np.stack([res.results[b]["out"] for b in range(NCORES)], axis=1)
    return np.ascontiguousarray(out.astype(np.float32))


# revision 72
# speedup vs baseline: 1.4238x; 1.0447x over previous
"""Trainium2 Bass kernel for nn_Attention_Module_15152644620833 (v5).

Reference computation (T=4096, B=8, D=1024, H=64, half=2048):
    q   = x[:half] @ Wq + bq            (half, B, H)
    k   = x @ Wk + bk                   (T, B, H)
    val = x @ Wv + bv                   (T, B, H)
    r   = posenc(T, D) @ Wr + br        (T, H)
    scores[b] = q[b] @ (k[b] + r).T + bias[b][None, :]
        where bias[b][m] = sum(u) * k[m,b,:].sum() + sum(v) * r[m,:].sum()
    causal mask on first `half` key positions, softmax over all T keys,
    out = attn @ val                    (half, B, H)

Sharding: data-parallel over batch, one batch element per NeuronCore.

v5 design (single-sweep pipeline, no collective):
  * r is computed locally on every core with zero communication.  Because
    pe[t] is sinusoidal, r.T for key chunk c equals W'_c.T @ pe0.T where
    pe0 is the FIRST 512-row block of the positional encoding and W'_c is
    Wr with its (sin, cos) row pairs rotated by the chunk phase
    phi_i = 512*c*div_i (host-side weight prep, exact identity).  Chunk
    pairs are packed into one 128-wide lhsT so the 8 chunks cost only
    4x8 matmuls; +br rides a K=1 matmul with a ones row.  pe0/W' are fp16
    (|pe|<=1; rel err ~6e-4 -> negligible vs the 2e-2 gate).
  * K2 (128, T) holds r.T on rows 0:64 (pair row 0:64 drains via DVE, the
    odd-chunk half via a partition-shift DMA) and k.T on rows 64:128;
    q2T holds [q; q] so scoresT = K2[:,msl].T @ q2T in one K=128 matmul.
  * key bias sum(u)*k_sum + sum(v)*r_sum folds multiplicatively into the
    val tiles (exp(s+b) = exp(s)*eb; eb scales both the val columns and
    the denominator ones column, so attn is unchanged, exactly).
  * causal mask: exp runs unmasked (|s|<~70 is fp32-safe); masked entries
    of diagonal-tile exp results are zeroed by a Pool affine_select, so
    the PE never runs mask matmuls.
  * attention is fused into the k/v streaming loop: when x chunk c lands,
    its projections run and every (query-chunk, key-tile) pair that just
    became computable is scored immediately.  Scores run in 2-tile groups
    so each activation-engine exp covers 1024 columns (the per-op PSUM
    access penalty is ~143ns, so per-tile exps would make ACT the
    bottleneck).  attnval accumulates per (iter, tq) event in a PSUM bank,
    then DVE-accumulates into an SBUF oT accumulator.  Score -> exp ->
    attnval is software-pipelined 2 groups deep so the in-order PE queue
    never waits on the activation engine.
  * DMA: the SP ring carries only the big loads in critical order
    (wkv, wqq, pe0, wrot, x chunks); all small transfers ride the ACT
    ring (each DMA costs ~650ns of ring-queue time regardless of size).
  * a dozen warmup matmuls on scratch data ramp the PE DVFS pstate to
    full clock before the first real matmul arrives.
"""

import math

import numpy as np

T, B, D, H = 4096, 8, 1024, 64
HALF = T // 2
P = 128
DC = D // P          # 8 d-chunks
NCH = T // 512       # 8 key/x chunks of 512
NTQ = HALF // 512    # 4 query chunks of 512
MT = T // P          # 32 key tiles of 128
NCORES = 8
LAGG = 3             # software-pipeline lag, in 2-tile score groups

_CACHE = {}


def _div_vec():
    return np.exp(
        np.arange(0, D, 2, dtype=np.float64) * (-(math.log(10000.0) / D))
    )


def _pe0T():
    """First 512 rows of the sin/cos positional encoding, transposed
    to (D, 512)."""
    div = _div_vec()
    ang = np.arange(512, dtype=np.float64)[:, None] * div[None, :]
    pe0 = np.stack([np.sin(ang), np.cos(ang)], axis=-1).reshape(512, D)
    return np.ascontiguousarray(pe0.T)


def _rotate_wr(Wr):
    """W'_c for every chunk c, packed in pairs (g, g+4): (D, 512) with
    columns [g*128 : g*128+64] = W'_g and [g*128+64 : (g+1)*128] =
    W'_{g+4}.  The early chunk of each pair lands on PSUM rows 0:64
    (lane-aligned DVE drain, available immediately); the late chunk's
    partition-shift DMA can wait for a crack in the x-stream.

    Identity: r[512c+s, h] = sum_d pe0[s, d] * W'_c[d, h]  (+ br), where
    W'_c rotates each (sin,cos) row pair of Wr by phi_i = 512*c*div_i.
    """
    div = _div_vec()
    Wr = np.asarray(Wr, dtype=np.float64)
    We, Wo = Wr[0::2, :], Wr[1::2, :]

    def _wc(c):
        phi = 512.0 * c * div
        cph, sph = np.cos(phi)[:, None], np.sin(phi)[:, None]
        Wc = np.empty((D, H), dtype=np.float64)
        Wc[0::2, :] = cph * We - sph * Wo
        Wc[1::2, :] = sph * We + cph * Wo
        return Wc

    cols = []
    for g in range(4):
        cols += [_wc(g), _wc(g + 4)]
    return np.ascontiguousarray(np.concatenate(cols, axis=1))


def _attn_events(c):
    """Events for iter c: list of (tq, [(mt, rel), ...]).  rel is the
    diagonal-tile offset (None for fully-unmasked tiles)."""
    if c < NTQ:
        tiles = []
        for kc in range(c):
            tiles += [(4 * kc + j, None) for j in range(4)]
        tiles += [(4 * c + j, j) for j in range(4)]
        return [(c, tiles)]
    return [(tq, [(4 * c + j, None) for j in range(4)]) for tq in range(NTQ)]


def _build_module():
    import concourse.bacc as bacc
    import concourse.bass_isa as bass_isa
    import concourse.mybir as mybir
    from concourse.masks import make_identity
    from concourse.tile import TileContext

    f32 = mybir.dt.float32
    f32r = mybir.dt.float32r
    f16 = mybir.dt.float16
    Exp = mybir.ActivationFunctionType.Exp

    nc = bacc.Bacc(num_devices=NCORES)

    xT_h = nc.dram_tensor("xT", [D, T], f32r, kind="ExternalInput")
    pe0_h = nc.dram_tensor("pe0", [D, 512], f16, kind="ExternalInput")
    wrot_h = nc.dram_tensor("wrot", [D, 512], f16, kind="ExternalInput")
    wkva_h = nc.dram_tensor("wkva", [D, 2 * H], f32r, kind="ExternalInput")
    wkvb_h = nc.dram_tensor("wkvb", [D, 2 * H], f32r, kind="ExternalInput")
    wqq_h = nc.dram_tensor("wqq", [D, 2 * H], f32r, kind="ExternalInput")
    # one combined small-constant tensor (a single early DMA; tiny DMAs
    # on a saturated engine otherwise wait ~30us for a crack in the
    # x-stream): cols [bkvA 0:1 | bkvB 1:2 | bqq 2:3 | uvcA 3:7 |
    # uvcB 7:11 | br2 11:12]
    sm_h = nc.dram_tensor("smalls", [2 * H, 12], f32, kind="ExternalInput")
    out_h = nc.dram_tensor("out", [HALF, H], f32, kind="ExternalOutput")

    xT_r = xT_h[:, :].rearrange("(c p) t -> p c t", p=P)       # (128, 8, T)
    pe0_r = pe0_h[:, :].rearrange("(c p) s -> p c s", p=P)     # (128, 8, 512)
    wrot_r = wrot_h[:, :].rearrange("(c p) h -> p c h", p=P)
    wkva_r = wkva_h[:, :].rearrange("(c p) h -> p c h", p=P)
    wkvb_r = wkvb_h[:, :].rearrange("(c p) h -> p c h", p=P)
    wqq_r = wqq_h[:, :].rearrange("(c p) h -> p c h", p=P)
    out_r = out_h[:, :].rearrange("(g p) h -> p g h", p=P)     # (128, 16, 64)

    with TileContext(nc) as tc, tc.tile_pool(name="persist", bufs=1) as persist:

        def _tile(shape, name, dt=f32):
            return persist.tile(shape, dt, name=name)

        # ---- persistent SBUF ------------------------------------------
        wkva_sb = _tile([P, DC, 2 * H], "wkva_sb", f32r)
        wkvb_sb = _tile([P, DC, 2 * H], "wkvb_sb", f32r)
        wqq_sb = _tile([P, DC, 2 * H], "wqq_sb", f32r)
        wrot_sb = _tile([P, DC, 4 * P], "wrot_sb", f16)
        pe0_sb = _tile([P, DC, 512], "pe0_sb", f16)
        sm_sb = _tile([2 * H, 12], "sm_sb")
        uv_colA = _tile([2 * H, 4], "uv_colA", f32r)
        uv_colB = _tile([2 * H, 4], "uv_colB", f32r)
        id_sb = _tile([P, P], "id_sb", f32r)
        warm_sb = _tile([P, 512], "warm_sb", f32r)
        dmask = _tile([P, 4, 512], "dmask", f32r)  # diag-tile 0/1 masks
        K2 = _tile([P, T], "K2", f32r)         # 0:64 r.T+br, 64:128 k.T+bk
        q2T = _tile([P, HALF], "q2T", f32r)    # rows 0:64 and 64:128 = q.T
        valaug = _tile([P, MT, H + 1], "valaug", f32r)
        ebias = _tile([P, MT], "ebias")
        oT_sb = _tile([H + 1, NTQ, 512], "oT_sb")
        outall = _tile([P, HALF // P, H], "outall")

        # ---- DMA queue: SP ring in critical order ----------------------
        nc.sync.dma_start(sm_sb[:], sm_h[:, :])
        nc.sync.dma_start(wkva_sb[:], wkva_r)

        with (
            tc.tile_pool(name="xstream", bufs=3) as xpool,
            tc.tile_pool(name="expool", bufs=5) as expool,
            tc.tile_pool(name="pshare", bufs=4, space="PSUM") as pshare,
            tc.tile_pool(name="psc", bufs=2, space="PSUM") as psc,
        ):
            # ---- constants built on device ----------------------------
            with tc.tile_pool(name="setupf", bufs=1) as setupf:
                warmf = setupf.tile([P, 512], f32, name="warmf")
                nc.gpsimd.memset(warmf[:], 0.125)
                nc.vector.tensor_copy(warm_sb[:], warmf[:])
                idf = setupf.tile([P, P], f32, name="idf")
                make_identity(nc, idf[:])
                nc.vector.tensor_copy(id_sb[:], idf[:])
                onesf = setupf.tile([P, MT], f32, name="onesf")
                nc.gpsimd.memset(onesf[:], 1.0)
                nc.vector.tensor_copy(valaug[:, :, H], onesf[:, 0:MT])
                # 0/1 masks for the four diagonal-tile offsets (keep where
                # key p <= query s, i.e. s - p - 128*rel >= 0)
                dmf = setupf.tile([P, 4, 512], f32, name="dmf")
                nc.gpsimd.memset(dmf[:], 1.0)
                for rel in range(4):
                    nc.gpsimd.affine_select(
                        out=dmf[:, rel, :], in_=dmf[:, rel, :],
                        compare_op=mybir.AluOpType.is_ge, fill=0.0,
                        base=-P * rel, pattern=[[1, 512]],
                        channel_multiplier=-1,
                    )
                nc.vector.tensor_copy(dmask[:], dmf[:])

            # PE pstate warmup: harmless matmuls on scratch data, ready
            # long before the first real operand DMA completes.
            wp = pshare.tile([P, 512], f32, name="wp", tag="sh")
            for _ in range(21):
                nc.tensor.matmul(
                    wp[:], warm_sb[:, 0:P], warm_sb[:], start=True, stop=True
                )

            # key-bias columns (sum(u)/sum(v) are host-computed scalars in
            # the smalls tensor): chunks 0-3 rows [r; k] -> [sum(v);
            # sum(u)], chunks 4-7 rows [k; r] -> [sum(u); sum(v)]
            nc.vector.tensor_copy(uv_colA[:], sm_sb[:, 3:7])
            nc.vector.tensor_copy(uv_colB[:], sm_sb[:, 7:11])

            def _r_chunks():
                # r.T chunks, local, no collective.  Pair (g, g+4) in one
                # 128-wide lhsT.  K2's row layout flips per chunk half
                # ([r; k] for chunks 0-3, [k; r] for 4-7 -- valid because
                # q2T is [q; q], so the score contraction is row-order-
                # free), which makes every PSUM drain lane-aligned: no
                # partition-shift DMAs anywhere.
                for g in range(4):
                    rp = pshare.tile([P, 512], f32, name="rp", tag="sh")
                    for dc in range(DC):
                        nc.tensor.matmul(
                            rp[:], wrot_sb[:, dc, g * P : (g + 1) * P],
                            pe0_sb[:, dc, :],
                            start=(dc == 0), stop=(dc == DC - 1),
                        )
                    nc.vector.tensor_scalar_add(
                        K2[0:H, g * 512 : (g + 1) * 512], rp[0:H, :],
                        sm_sb[0:H, 11:12],
                    )
                    nc.vector.tensor_scalar_add(
                        K2[H:P, (g + 4) * 512 : (g + 5) * 512], rp[H:P, :],
                        sm_sb[H : 2 * H, 11:12],
                    )

            # ---- streaming sweep: projections + fused attention -------
            ev_state = {}     # (c, tq) -> [av_tile, tiles_done, total]
            tq_seen = set()
            pend = []

            def emit_output(tq):
                for j in range(4):
                    tp = pshare.tile([P, 512], f32, name="tp",
                                     tag="sh")[:, 0 : H + 1]
                    nc.tensor.transpose(
                        tp[:], oT_sb[:, tq, j * P : (j + 1) * P],
                        id_sb[0 : H + 1, 0 : H + 1].bitcast(f32),
                    )
                    inv = xpool.tile([P, 1], f32, name="inv", tag="inv",
                                     bufs=2)
                    nc.vector.reciprocal(inv[:], tp[:, H : H + 1])
                    nc.vector.tensor_scalar_mul(
                        outall[:, tq * 4 + j, :], tp[:, 0:H], inv[:]
                    )
                nc.scalar.dma_start(
                    out_r[:, tq * 4 : (tq + 1) * 4, :],
                    outall[:, tq * 4 : (tq + 1) * 4, :],
                )

            def pop_group():
                c, tq, grp, ex2, total = pend.pop(0)
                key = (c, tq)
                st = ev_state.get(key)
                if st is None:
                    av = pshare.tile([P, 512], f32, name="av", tag="sh")
                    st = ev_state[key] = [av, 0, total]
                for i, (mt, rel) in enumerate(grp):
                    nc.tensor.matmul(
                        st[0][0 : H + 1, :], valaug[:, mt, :], ex2[:, i, :],
                        start=(st[1] == 0), stop=(st[1] == total - 1),
                    )
                    st[1] += 1
                if st[1] == total:
                    last = c == NCH - 1 and tq == NTQ - 1
                    if last:
                        # final event: accumulate per 128-column block so
                        # the output transposes pipeline with the adds
                        for j in range(4):
                            jsl = slice(j * P, (j + 1) * P)
                            nc.vector.tensor_add(
                                oT_sb[:, tq, jsl], oT_sb[:, tq, jsl],
                                st[0][0 : H + 1, jsl],
                            )
                    elif tq in tq_seen:
                        nc.vector.tensor_add(
                            oT_sb[:, tq, :], oT_sb[:, tq, :],
                            st[0][0 : H + 1, :],
                        )
                    else:
                        nc.vector.tensor_copy(
                            oT_sb[:, tq, :], st[0][0 : H + 1, :]
                        )
                        tq_seen.add(tq)
                    # delay each output stage one event so its transposes
                    # never wait on the just-issued DVE accumulation
                    if c == NCH - 1 and tq > 0:
                        emit_output(tq - 1)

            # x-chunk loads: 2-ahead prefetch so the WAR wait on a reused
            # slot is emitted after that slot's readers (emitting all 8
            # upfront serializes the stream against future readers)
            xts = {}

            def _prefetch(c):
                if c >= NCH or c in xts:
                    return
                xt = xpool.tile([P, DC, 512], f32r, name="xt", tag="xt")
                nc.sync.dma_start(xt[:], xT_r[:, :, c * 512 : (c + 1) * 512])
                xts[c] = xt

            # ring order: pe0/wrot ahead of xt0 -- r fills the PE while
            # the x stream starts; wqq behind xt0 (q0 is emitted after r)
            nc.sync.dma_start(pe0_sb[:], pe0_r)
            nc.sync.dma_start(wrot_sb[:], wrot_r)
            _prefetch(0)
            nc.sync.dma_start(wqq_sb[:], wqq_r)
            _prefetch(1)
            vstages = {}

            # bracket b: projections of chunk b + attention of chunk b-1.
            # The one-iter shift means every PE instruction's inputs were
            # DVE-drained a full bracket earlier -> no intra-iter stalls.
            def _kv_proj(c):
                sl = slice(c * 512, (c + 1) * 512)
                lo = c < NTQ          # chunk half: [r; k] rows vs [k; r]
                wkv_sb = wkva_sb if lo else wkvb_sb
                var = 0 if lo else 1
                kvp = pshare.tile([P, 512], f32, name="kvp", tag="sh")
                for dc in range(DC):
                    nc.tensor.matmul(
                        kvp[:], wkv_sb[:, dc, :], xts[c][:, dc, :],
                        start=(dc == 0), stop=(dc == DC - 1),
                    )
                # chunks 0-3: kvp rows [v; k], k -> K2 rows 64:128;
                # chunks 4-7: kvp rows [k; v], k -> K2 rows 0:64.
                # All drains lane-aligned.
                vstage = xpool.tile([P, 512], f32r, name="vstage",
                                    tag="vst", bufs=2)
                vstages[c] = vstage
                vsl = slice(0, H) if lo else slice(H, P)
                ksl = slice(H, P) if lo else slice(0, H)
                nc.vector.tensor_scalar_add(
                    vstage[vsl, :], kvp[vsl, :], sm_sb[vsl, var : var + 1]
                )
                nc.vector.tensor_scalar_add(
                    K2[ksl, sl], kvp[ksl, :], sm_sb[ksl, var : var + 1]
                )

            def _q_proj(c):
                sl = slice(c * 512, (c + 1) * 512)
                qp = pshare.tile([P, 512], f32, name="qp", tag="sh")
                for dc in range(DC):
                    nc.tensor.matmul(
                        qp[:], wqq_sb[:, dc, :], xts[c][:, dc, :],
                        start=(dc == 0), stop=(dc == DC - 1),
                    )
                nc.vector.tensor_scalar_add(
                    q2T[:, sl], qp[:], sm_sb[:, 2:3]
                )

            for b in range(NCH + 1):
                if b == 0:
                    # bracket 0: r (pe0/wrot land before xt0), then kv0/q0
                    _prefetch(2)
                    _r_chunks()
                    _kv_proj(0)
                    _q_proj(0)
                    continue
                if b < NCH:
                    if b == 2:
                        # wkvb isn't needed until bracket 4; ride it ahead
                        # of xt4 only
                        nc.sync.dma_start(wkvb_sb[:], wkvb_r)
                    _prefetch(b + 2)
                    _kv_proj(b)
                    if b < NTQ:
                        _q_proj(b)
                c = b - 1
                # v transposes + key-bias matmuls share one PSUM tile
                # (disjoint column regions)
                vb = pshare.tile([P, 512], f32, name="vb", tag="sh")
                vstage = vstages.pop(c)
                cvsl = slice(0, H) if c < NTQ else slice(H, P)
                cvar = 0 if c < NTQ else 1
                for j in range(4):
                    nc.tensor.transpose(
                        vb[:, j * H : (j + 1) * H].bitcast(f32r),
                        vstage[cvsl, j * P : (j + 1) * P],
                        id_sb[cvsl, cvsl],
                    )
                uv_col = uv_colA if cvar == 0 else uv_colB
                for j in range(4):
                    mt = c * 4 + j
                    msl = slice(mt * P, (mt + 1) * P)
                    nc.tensor.matmul(
                        vb[:, 4 * H + 4 * j : 4 * H + 4 * j + 4], K2[:, msl],
                        uv_col[:], start=True, stop=True,
                    )
                nc.scalar.activation(
                    ebias[:, c * 4 : (c + 1) * 4],
                    vb[:, 4 * H : 4 * H + 16 : 4], Exp,
                )
                for j in range(4):
                    mt = c * 4 + j
                    nc.vector.tensor_scalar_mul(
                        valaug[:, mt, 0:H],
                        vb[:, j * H : (j + 1) * H].bitcast(f32r),
                        ebias[:, mt : mt + 1],
                    )
                    nc.vector.tensor_copy(
                        valaug[:, mt, H : H + 1], ebias[:, mt : mt + 1]
                    )

                # fused attention for everything unlocked by chunk c
                for tq, tiles in _attn_events(c):
                    tsl = slice(tq * 512, (tq + 1) * 512)
                    total = len(tiles)
                    for gi in range(0, total, 2):
                        grp = tiles[gi : gi + 2]
                        sp2 = psc.tile([P, 2, 512], f32, name="sp", tag="sc")
                        for i, (mt, rel) in enumerate(grp):
                            msl = slice(mt * P, (mt + 1) * P)
                            nc.tensor.matmul(
                                sp2[:, i, :], K2[:, msl], q2T[:, tsl],
                                start=True, stop=True,
                            )
                        ex2 = expool.tile([P, 2, 512], f32r, name="ex",
                                          tag="ex")
                        nc.scalar.activation(ex2[:], sp2[:], Exp)
                        for i, (mt, rel) in enumerate(grp):
                            if rel is not None:
                                # zero masked entries on the idle Pool
                                # engine (f32r ALU output is rounded, so
                                # the attnval matmul accepts it)
                                nc.gpsimd.tensor_mul(
                                    ex2[:, i, :], ex2[:, i, :],
                                    dmask[:, rel, :],
                                )
                        pend.append((c, tq, grp, ex2, total))
                        if len(pend) > LAGG:
                            pop_group()
            while pend:
                pop_group()
            emit_output(NTQ - 1)

    nc.compile()
    return nc


def _get_module():
    if "nc" not in _CACHE:
        _CACHE["nc"] = _build_module()
    return _CACHE["nc"]


def _make_in_maps(inputs):
    inp = np.asarray(inputs["inp_data"], dtype=np.float32)
    Wq = np.asarray(inputs["Wq"], dtype=np.float32)
    bq = np.asarray(inputs["bq"], dtype=np.float32)
    Wk = np.asarray(inputs["Wk"], dtype=np.float32)
    bk = np.asarray(inputs["bk"], dtype=np.float32)
    Wv = np.asarray(inputs["Wv"], dtype=np.float32)
    bv = np.asarray(inputs["bv"], dtype=np.float32)
    Wr = np.asarray(inputs["Wr"], dtype=np.float32)
    br = np.asarray(inputs["br"], dtype=np.float32)
    u = np.asarray(inputs["u"], dtype=np.float32)
    v = np.asarray(inputs["v"], dtype=np.float32)

    if "pe0" not in _CACHE:
        _CACHE["pe0"] = np.ascontiguousarray(_pe0T().astype(np.float16))
    us, vs = np.float32(u.sum()), np.float32(v.sum())
    sm = np.zeros((2 * H, 12), dtype=np.float32)
    sm[:, 0] = np.concatenate([bv, bk])       # bkv, chunks 0-3 ([v; k])
    sm[:, 1] = np.concatenate([bk, bv])       # bkv, chunks 4-7 ([k; v])
    sm[:, 2] = np.concatenate([bq, bq])       # bqq
    sm[0:H, 3:7], sm[H:, 3:7] = vs, us        # uv col, chunks 0-3 [r; k]
    sm[0:H, 7:11], sm[H:, 7:11] = us, vs      # uv col, chunks 4-7 [k; r]
    sm[0:H, 11] = br                          # br on both pair halves
    sm[H:, 11] = br
    common = {
        "pe0": _CACHE["pe0"],
        "wrot": np.ascontiguousarray(_rotate_wr(Wr).astype(np.float16)),
        "wkva": np.ascontiguousarray(np.concatenate([Wv, Wk], axis=1)),
        "wkvb": np.ascontiguousarray(np.concatenate([Wk, Wv], axis=1)),
        "wqq": np.ascontiguousarray(np.concatenate([Wq, Wq], axis=1)),
        "smalls": sm,
    }
    in_maps = []
    for b in range(NCORES):
        m = {"xT": np.ascontiguousarray(inp[:, b, :].T)}
        m.update(common)
        in_maps.append(m)
    return in_maps


def _run(in_maps, trace=False):
    from concourse.bass_utils import run_bass_kernel_spmd

    nc = _get_module()
    return run_bass_kernel_spmd(
        nc, in_maps, core_ids=list(range(NCORES)), trace=trace
    )


def _timed_run(in_maps, iters=5, reps=1):
    """Replicates bass2jax.run_bass_via_pjrt's multi-core path, but keeps the
    jitted callable and device-resident inputs so repeated executions can be
    wall-clock timed (no NTFF profiling is available through the axon client).
    """
    import time

    import jax
    import concourse.mybir as mybir
    from concourse.bass2jax import (
        _bass_exec_p,
        install_neuronx_cc_hook,
        partition_id_tensor,
    )
    from jax.experimental.shard_map import shard_map
    from jax.sharding import Mesh, NamedSharding, PartitionSpec

    nc = _get_module()
    install_neuronx_cc_hook()
    partition_name = nc.partition_id_tensor.name if nc.partition_id_tensor else None

    in_names, out_names, out_avals, zero_shapes = [], [], [], []
    for alloc in nc.m.functions[0].allocations:
        if not isinstance(alloc, mybir.MemoryLocationSet):
            continue
        name = alloc.memorylocations[0].name
        if alloc.kind == "ExternalInput":
            if name != partition_name:
                in_names.append(name)
        elif alloc.kind == "ExternalOutput":
            out_names.append(name)
            shape = tuple(alloc.tensor_shape)
            dtype = mybir.dt.np(alloc.dtype)
            out_avals.append(jax.core.ShapedArray(shape, dtype))
            zero_shapes.append((shape, dtype))
    n_params = len(in_names)
    all_names = in_names + out_names
    if partition_name is not None:
        all_names = all_names + [partition_name]
    donate = tuple(range(n_params, n_params + len(out_names)))

    def _body(*args):
        operands = list(args)
        if partition_name is not None:
            operands.append(partition_id_tensor())
        outs = _bass_exec_p.bind(
            *operands,
            out_avals=tuple(out_avals),
            in_names=tuple(all_names),
            out_names=tuple(out_names),
            lowering_input_output_aliases=(),
            sim_require_finite=True,
            sim_require_nnan=True,
            nc=nc,
        )
        return tuple(outs)

    devices = jax.devices()[:NCORES]
    mesh = Mesh(np.asarray(devices), ("core",))
    spec = PartitionSpec("core")
    in_specs = (spec,) * (n_params + len(out_names))
    sharded = jax.jit(
        shard_map(
            _body, mesh=mesh, in_specs=in_specs,
            out_specs=(spec,) * len(out_names), check_rep=False,
        ),
        donate_argnums=donate,
        keep_unused=True,
    )
    sharding = NamedSharding(mesh, spec)
    concat_in = [
        jax.device_put(
            np.concatenate([in_maps[c][nm] for c in range(NCORES)], axis=0), sharding
        )
        for nm in in_names
    ]

    def zeros():
        return [
            jax.device_put(np.zeros((NCORES * s[0], *s[1:]), d), sharding)
            for (s, d) in zero_shapes
        ]

    out = sharded(*concat_in, *zeros())
    jax.block_until_ready(out)
    times = []
    for _ in range(iters):
        zs = zeros()
        jax.block_until_ready(zs)
        t0 = time.perf_counter()
        out = sharded(*concat_in, *zs)
        jax.block_until_ready(out)
        times.append(time.perf_counter() - t0)
    results = {
        nm: np.asarray(out[i]).reshape(NCORES, *out_avals[i].shape)
        for i, nm in enumerate(out_names)
    }
    return results, times


def kernel(**inputs) -> np.ndarray:
    in_maps = _make_in_maps(inputs)
    res = _run(in_maps, trace=False)
    out = np.stack([res.results[b]["out"] for b in range(NCORES)], axis=1)
    return np.ascontiguousarray(out.astype(np.float32))


# revision 91
# speedup vs baseline: 1.5146x; 1.0638x over previous
"""Trainium2 Bass kernel for nn_Attention_Module_15152644620833 (v5).

Reference computation (T=4096, B=8, D=1024, H=64, half=2048):
    q   = x[:half] @ Wq + bq            (half, B, H)
    k   = x @ Wk + bk                   (T, B, H)
    val = x @ Wv + bv                   (T, B, H)
    r   = posenc(T, D) @ Wr + br        (T, H)
    scores[b] = q[b] @ (k[b] + r).T + bias[b][None, :]
        where bias[b][m] = sum(u) * k[m,b,:].sum() + sum(v) * r[m,:].sum()
    causal mask on first `half` key positions, softmax over all T keys,
    out = attn @ val                    (half, B, H)

Sharding: data-parallel over batch, one batch element per NeuronCore.

v5 design (single-sweep pipeline, no collective):
  * r is computed locally on every core with zero communication.  Because
    pe[t] is sinusoidal, r.T for key chunk c equals W'_c.T @ pe0.T where
    pe0 is the FIRST 512-row block of the positional encoding and W'_c is
    Wr with its (sin, cos) row pairs rotated by the chunk phase
    phi_i = 512*c*div_i (host-side weight prep, exact identity).  Chunk
    pairs are packed into one 128-wide lhsT so the 8 chunks cost only
    4x8 matmuls; +br rides a K=1 matmul with a ones row.  pe0/W' are fp16
    (|pe|<=1; rel err ~6e-4 -> negligible vs the 2e-2 gate).
  * K2 (128, T) holds r.T on rows 0:64 (pair row 0:64 drains via DVE, the
    odd-chunk half via a partition-shift DMA) and k.T on rows 64:128;
    q2T holds [q; q] so scoresT = K2[:,msl].T @ q2T in one K=128 matmul.
  * key bias sum(u)*k_sum + sum(v)*r_sum folds multiplicatively into the
    val tiles (exp(s+b) = exp(s)*eb; eb scales both the val columns and
    the denominator ones column, so attn is unchanged, exactly).
  * causal mask: exp runs unmasked (|s|<~70 is fp32-safe); masked entries
    of diagonal-tile exp results are zeroed by a Pool affine_select, so
    the PE never runs mask matmuls.
  * attention is fused into the k/v streaming loop: when x chunk c lands,
    its projections run and every (query-chunk, key-tile) pair that just
    became computable is scored immediately.  Scores run in 2-tile groups
    so each activation-engine exp covers 1024 columns (the per-op PSUM
    access penalty is ~143ns, so per-tile exps would make ACT the
    bottleneck).  attnval accumulates per (iter, tq) event in a PSUM bank,
    then DVE-accumulates into an SBUF oT accumulator.  Score -> exp ->
    attnval is software-pipelined 2 groups deep so the in-order PE queue
    never waits on the activation engine.
  * DMA: the SP ring carries only the big loads in critical order
    (wkv, wqq, pe0, wrot, x chunks); all small transfers ride the ACT
    ring (each DMA costs ~650ns of ring-queue time regardless of size).
  * a dozen warmup matmuls on scratch data ramp the PE DVFS pstate to
    full clock before the first real matmul arrives.
"""

import math

import numpy as np

T, B, D, H = 4096, 8, 1024, 64
HALF = T // 2
P = 128
DC = D // P          # 8 d-chunks
NCH = T // 512       # 8 key/x chunks of 512
NTQ = HALF // 512    # 4 query chunks of 512
MT = T // P          # 32 key tiles of 128
NCORES = 8
LAGG = 5             # software-pipeline lag, in 2-tile score groups

_CACHE = {}


def _div_vec():
    return np.exp(
        np.arange(0, D, 2, dtype=np.float64) * (-(math.log(10000.0) / D))
    )


def _pe0T():
    """First 512 rows of the sin/cos positional encoding, transposed
    to (D, 512)."""
    div = _div_vec()
    ang = np.arange(512, dtype=np.float64)[:, None] * div[None, :]
    pe0 = np.stack([np.sin(ang), np.cos(ang)], axis=-1).reshape(512, D)
    return np.ascontiguousarray(pe0.T)


def _rotate_wr(Wr):
    """W'_c for every chunk c, packed in pairs (g, g+4): (D, 512) with
    columns [g*128 : g*128+64] = W'_g and [g*128+64 : (g+1)*128] =
    W'_{g+4}.  The early chunk of each pair lands on PSUM rows 0:64
    (lane-aligned DVE drain, available immediately); the late chunk's
    partition-shift DMA can wait for a crack in the x-stream.

    Identity: r[512c+s, h] = sum_d pe0[s, d] * W'_c[d, h]  (+ br), where
    W'_c rotates each (sin,cos) row pair of Wr by phi_i = 512*c*div_i.
    """
    div = _div_vec()
    Wr = np.asarray(Wr, dtype=np.float64)
    We, Wo = Wr[0::2, :], Wr[1::2, :]

    def _wc(c):
        phi = 512.0 * c * div
        cph, sph = np.cos(phi)[:, None], np.sin(phi)[:, None]
        Wc = np.empty((D, H), dtype=np.float64)
        Wc[0::2, :] = cph * We - sph * Wo
        Wc[1::2, :] = sph * We + cph * Wo
        return Wc

    cols = []
    for g in range(4):
        cols += [_wc(g), _wc(g + 4)]
    return np.ascontiguousarray(np.concatenate(cols, axis=1))


def _attn_events(c):
    """Events for iter c: list of (tq, [(mt, rel), ...]).  rel is the
    diagonal-tile offset (None for fully-unmasked tiles)."""
    if c < NTQ:
        tiles = []
        for kc in range(c):
            tiles += [(4 * kc + j, None) for j in range(4)]
        tiles += [(4 * c + j, j) for j in range(4)]
        return [(c, tiles)]
    return [(tq, [(4 * c + j, None) for j in range(4)]) for tq in range(NTQ)]


def _build_module():
    import concourse.bacc as bacc
    import concourse.bass_isa as bass_isa
    import concourse.mybir as mybir
    from concourse.masks import make_identity
    from concourse.tile import TileContext

    f32 = mybir.dt.float32
    f32r = mybir.dt.float32r
    f16 = mybir.dt.float16
    Exp = mybir.ActivationFunctionType.Exp

    nc = bacc.Bacc(num_devices=NCORES)

    xT_h = nc.dram_tensor("xT", [D, T], f32r, kind="ExternalInput")
    pe0_h = nc.dram_tensor("pe0", [D, 512], f16, kind="ExternalInput")
    wrot_h = nc.dram_tensor("wrot", [D, 512], f16, kind="ExternalInput")
    wkva_h = nc.dram_tensor("wkva", [D, 2 * H], f32r, kind="ExternalInput")
    wkvb_h = nc.dram_tensor("wkvb", [D, 2 * H], f32r, kind="ExternalInput")
    wqq_h = nc.dram_tensor("wqq", [D, 2 * H], f32r, kind="ExternalInput")
    # one combined small-constant tensor (a single early DMA; tiny DMAs
    # on a saturated engine otherwise wait ~30us for a crack in the
    # x-stream): cols [bkvA 0:1 | bkvB 1:2 | bqq 2:3 | uvcA 3:7 |
    # uvcB 7:11 | br2 11:12]
    sm_h = nc.dram_tensor("smalls", [2 * H, 12], f32, kind="ExternalInput")
    out_h = nc.dram_tensor("out", [HALF, H], f32, kind="ExternalOutput")

    xT_r = xT_h[:, :].rearrange("(c p) t -> p c t", p=P)       # (128, 8, T)
    pe0_r = pe0_h[:, :].rearrange("(c p) s -> p c s", p=P)     # (128, 8, 512)
    wrot_r = wrot_h[:, :].rearrange("(c p) h -> p c h", p=P)
    wkva_r = wkva_h[:, :].rearrange("(c p) h -> p c h", p=P)
    wkvb_r = wkvb_h[:, :].rearrange("(c p) h -> p c h", p=P)
    wqq_r = wqq_h[:, :].rearrange("(c p) h -> p c h", p=P)
    out_r = out_h[:, :].rearrange("(g p) h -> p g h", p=P)     # (128, 16, 64)

    with TileContext(nc) as tc, tc.tile_pool(name="persist", bufs=1) as persist:

        def _tile(shape, name, dt=f32):
            return persist.tile(shape, dt, name=name)

        # ---- persistent SBUF ------------------------------------------
        wkva_sb = _tile([P, DC, 2 * H], "wkva_sb", f32r)
        wkvb_sb = _tile([P, DC, 2 * H], "wkvb_sb", f32r)
        wqq_sb = _tile([P, DC, 2 * H], "wqq_sb", f32r)
        wrot_sb = _tile([P, DC, 4 * P], "wrot_sb", f16)
        pe0_sb = _tile([P, DC, 512], "pe0_sb", f16)
        sm_sb = _tile([2 * H, 12], "sm_sb")
        uv_colA = _tile([2 * H, 4], "uv_colA", f32r)
        uv_colB = _tile([2 * H, 4], "uv_colB", f32r)
        id_sb = _tile([P, P], "id_sb", f32r)
        warm_sb = _tile([P, 512], "warm_sb", f32r)
        dmask = _tile([P, 4, 512], "dmask", f32r)  # diag-tile 0/1 masks
        K2 = _tile([P, T], "K2", f32r)         # 0:64 r.T+br, 64:128 k.T+bk
        q2T = _tile([P, HALF], "q2T", f32r)    # rows 0:64 and 64:128 = q.T
        valaug = _tile([P, MT, H + 1], "valaug", f32r)
        ebias = _tile([P, MT], "ebias")
        oT_sb = _tile([H + 1, NTQ, 512], "oT_sb")
        outall = _tile([P, HALF // P, H], "outall")

        # ---- DMA queue: SP ring in critical order.  These MUST be
        # emitted before anything that reads the tiles: Tile builds the
        # dependency graph from program order, so a read emitted ahead of
        # the write has no RAW edge and reads stale SBUF on the first run.
        nc.sync.dma_start(sm_sb[:], sm_h[:, :])
        nc.sync.dma_start(wkva_sb[:], wkva_r)
        nc.sync.dma_start(pe0_sb[:], pe0_r)
        nc.sync.dma_start(wrot_sb[:], wrot_r)

        with (
            tc.tile_pool(name="xstream", bufs=3) as xpool,
            tc.tile_pool(name="expool", bufs=7) as expool,
            tc.tile_pool(name="pshare", bufs=4, space="PSUM") as pshare,
            tc.tile_pool(name="psc", bufs=2, space="PSUM") as psc,
        ):
            # ---- constants built on device ----------------------------
            with tc.tile_pool(name="setupf", bufs=1) as setupf:
                warmf = setupf.tile([P, 512], f32, name="warmf")
                nc.gpsimd.memset(warmf[:], 0.125)
                nc.vector.tensor_copy(warm_sb[:], warmf[:])
                idf = setupf.tile([P, P], f32, name="idf")
                make_identity(nc, idf[:])
                nc.vector.tensor_copy(id_sb[:], idf[:])
                onesf = setupf.tile([P, MT], f32, name="onesf")
                nc.gpsimd.memset(onesf[:], 1.0)
                nc.vector.tensor_copy(valaug[:, :, H], onesf[:, 0:MT])
                # 0/1 masks for the four diagonal-tile offsets (keep where
                # key p <= query s, i.e. s - p - 128*rel >= 0)
                dmf = setupf.tile([P, 4, 512], f32, name="dmf")
                nc.gpsimd.memset(dmf[:], 1.0)
                for rel in range(4):
                    nc.gpsimd.affine_select(
                        out=dmf[:, rel, :], in_=dmf[:, rel, :],
                        compare_op=mybir.AluOpType.is_ge, fill=0.0,
                        base=-P * rel, pattern=[[1, 512]],
                        channel_multiplier=-1,
                    )
                nc.vector.tensor_copy(dmask[:], dmf[:])

            # PE pstate warmup: harmless matmuls on scratch data, ready
            # long before the first real operand DMA completes.
            wp = pshare.tile([P, 512], f32, name="wp", tag="sh")
            for _ in range(21):
                nc.tensor.matmul(
                    wp[:], warm_sb[:, 0:P], warm_sb[:], start=True, stop=True
                )

            # key-bias columns (sum(u)/sum(v) are host-computed scalars in
            # the smalls tensor): chunks 0-3 rows [r; k] -> [sum(v);
            # sum(u)], chunks 4-7 rows [k; r] -> [sum(u); sum(v)]
            nc.vector.tensor_copy(uv_colA[:], sm_sb[:, 3:7])
            nc.vector.tensor_copy(uv_colB[:], sm_sb[:, 7:11])

            def _r_group(g):
                # r.T chunk pair (g, g+4) in one 128-wide lhsT -- local,
                # no collective.  K2's row layout flips per chunk half
                # ([r; k] for chunks 0-3, [k; r] for 4-7 -- valid because
                # q2T is [q; q], so the score contraction is row-order-
                # free), which makes every PSUM drain lane-aligned: no
                # partition-shift DMAs anywhere.  Group g is emitted in
                # bracket g (needed first by bracket g+1), filling the
                # early x-stream DMA waits with useful work.
                rp = pshare.tile([P, 512], f32, name="rp", tag="sh")
                for dc in range(DC):
                    nc.tensor.matmul(
                        rp[:], wrot_sb[:, dc, g * P : (g + 1) * P],
                        pe0_sb[:, dc, :],
                        start=(dc == 0), stop=(dc == DC - 1),
                    )
                nc.vector.tensor_scalar_add(
                    K2[0:H, g * 512 : (g + 1) * 512], rp[0:H, :],
                    sm_sb[0:H, 11:12],
                )
                nc.vector.tensor_scalar_add(
                    K2[H:P, (g + 4) * 512 : (g + 5) * 512], rp[H:P, :],
                    sm_sb[H : 2 * H, 11:12],
                )

            # ---- streaming sweep: projections + fused attention -------
            ev_state = {}     # (c, tq) -> [av_tile, tiles_done, total]
            tq_seen = set()
            pend = []

            def emit_output(tq):
                for j in range(4):
                    tp = pshare.tile([P, 512], f32, name="tp",
                                     tag="sh")[:, 0 : H + 1]
                    nc.tensor.transpose(
                        tp[:], oT_sb[:, tq, j * P : (j + 1) * P],
                        id_sb[0 : H + 1, 0 : H + 1].bitcast(f32),
                    )
                    inv = xpool.tile([P, 1], f32, name="inv", tag="inv",
                                     bufs=2)
                    nc.vector.reciprocal(inv[:], tp[:, H : H + 1])
                    nc.vector.tensor_scalar_mul(
                        outall[:, tq * 4 + j, :], tp[:, 0:H], inv[:]
                    )
                for jj in range(2):
                    o0 = tq * 4 + 2 * jj
                    nc.scalar.dma_start(
                        out_r[:, o0 : o0 + 2, :], outall[:, o0 : o0 + 2, :]
                    )

            def pop_group():
                c, tq, grp, ex2, total = pend.pop(0)
                key = (c, tq)
                st = ev_state.get(key)
                if st is None:
                    av = pshare.tile([P, 512], f32, name="av", tag="sh")
                    st = ev_state[key] = [av, 0, total]
                for i, (mt, rel) in enumerate(grp):
                    nc.tensor.matmul(
                        st[0][0 : H + 1, :], valaug[:, mt, :], ex2[:, i, :],
                        start=(st[1] == 0), stop=(st[1] == total - 1),
                    )
                    st[1] += 1
                if st[1] == total:
                    last = c == NCH - 1 and tq == NTQ - 1
                    if last:
                        # final event: accumulate per 128-column block so
                        # the output transposes pipeline with the adds
                        for j in range(4):
                            jsl = slice(j * P, (j + 1) * P)
                            nc.vector.tensor_add(
                                oT_sb[:, tq, jsl], oT_sb[:, tq, jsl],
                                st[0][0 : H + 1, jsl],
                            )
                    elif tq in tq_seen:
                        nc.vector.tensor_add(
                            oT_sb[:, tq, :], oT_sb[:, tq, :],
                            st[0][0 : H + 1, :],
                        )
                    else:
                        nc.vector.tensor_copy(
                            oT_sb[:, tq, :], st[0][0 : H + 1, :]
                        )
                        tq_seen.add(tq)
                    # delay each output stage one event so its transposes
                    # never wait on the just-issued DVE accumulation
                    if c == NCH - 1 and tq > 0:
                        emit_output(tq - 1)

            # x-chunk loads: 2-ahead prefetch so the WAR wait on a reused
            # slot is emitted after that slot's readers (emitting all 8
            # upfront serializes the stream against future readers)
            xts = {}

            def _prefetch(c):
                if c >= NCH or c in xts:
                    return
                xt = xpool.tile([P, DC, 512], f32r, name="xt", tag="xt")
                # two half-column DMAs: the projection matmuls start on
                # the first half ~2.9us before the full chunk would land
                for h in range(2):
                    nc.sync.dma_start(
                        xt[:, :, h * 256 : (h + 1) * 256],
                        xT_r[:, :, c * 512 + h * 256 : c * 512 + (h + 1) * 256],
                    )
                xts[c] = xt

            _prefetch(0)
            nc.sync.dma_start(wqq_sb[:], wqq_r)
            _prefetch(1)
            vstages = {}

            # bracket b: projections of chunk b + attention of chunk b-1.
            # The one-iter shift means every PE instruction's inputs were
            # DVE-drained a full bracket earlier -> no intra-iter stalls.
            def _kv_proj(c):
                sl = slice(c * 512, (c + 1) * 512)
                lo = c < NTQ          # chunk half: [r; k] rows vs [k; r]
                wkv_sb = wkva_sb if lo else wkvb_sb
                var = 0 if lo else 1
                kvp = pshare.tile([P, 512], f32, name="kvp", tag="sh")
                for h in range(2):
                    hsl = slice(h * 256, (h + 1) * 256)
                    for dc in range(DC):
                        nc.tensor.matmul(
                            kvp[:, hsl], wkv_sb[:, dc, :],
                            xts[c][:, dc, hsl],
                            start=(dc == 0), stop=(dc == DC - 1),
                        )
                # chunks 0-3: kvp rows [v; k], k -> K2 rows 64:128;
                # chunks 4-7: kvp rows [k; v], k -> K2 rows 0:64.
                # All drains lane-aligned.
                vstage = xpool.tile([P, 512], f32r, name="vstage",
                                    tag="vst", bufs=2)
                vstages[c] = vstage
                vsl = slice(0, H) if lo else slice(H, P)
                ksl = slice(H, P) if lo else slice(0, H)
                nc.vector.tensor_scalar_add(
                    vstage[vsl, :], kvp[vsl, :], sm_sb[vsl, var : var + 1]
                )
                nc.vector.tensor_scalar_add(
                    K2[ksl, sl], kvp[ksl, :], sm_sb[ksl, var : var + 1]
                )

            def _q_proj(c):
                sl = slice(c * 512, (c + 1) * 512)
                qp = pshare.tile([P, 512], f32, name="qp", tag="sh")
                for h in range(2):
                    hsl = slice(h * 256, (h + 1) * 256)
                    for dc in range(DC):
                        nc.tensor.matmul(
                            qp[:, hsl], wqq_sb[:, dc, :],
                            xts[c][:, dc, hsl],
                            start=(dc == 0), stop=(dc == DC - 1),
                        )
                nc.vector.tensor_scalar_add(
                    q2T[:, sl], qp[:], sm_sb[:, 2:3]
                )

            for b in range(NCH + 1):
                if b == 0:
                    # bracket 0: r pair 0 (pe0/wrot land before xt0),
                    # then kv0/q0
                    _prefetch(2)
                    for g in range(4):
                        _r_group(g)
                    _kv_proj(0)
                    _q_proj(0)
                    continue
                if b < NCH:
                    if b == 2:
                        # wkvb isn't needed until bracket 4; ride it ahead
                        # of xt4 only
                        nc.sync.dma_start(wkvb_sb[:], wkvb_r)
                    _prefetch(b + 2)
                    _kv_proj(b)
                    if b < NTQ:
                        _q_proj(b)
                c = b - 1
                # v transposes and key-bias matmuls in natively-typed PSUM
                # tiles (write-bitcast APs break Tile's region aliasing ->
                # races on hardware)
                vtr = pshare.tile([P, 512], f32r, name="vtr", tag="sh")
                bp = pshare.tile([P, 512], f32, name="bp", tag="sh")
                vstage = vstages.pop(c)
                cvsl = slice(0, H) if c < NTQ else slice(H, P)
                cvar = 0 if c < NTQ else 1
                for j in range(4):
                    nc.tensor.transpose(
                        vtr[:, j * H : (j + 1) * H],
                        vstage[cvsl, j * P : (j + 1) * P],
                        id_sb[cvsl, cvsl],
                    )
                uv_col = uv_colA if cvar == 0 else uv_colB
                for j in range(4):
                    mt = c * 4 + j
                    msl = slice(mt * P, (mt + 1) * P)
                    nc.tensor.matmul(
                        bp[:, 4 * j : 4 * j + 4], K2[:, msl],
                        uv_col[:], start=True, stop=True,
                    )
                nc.scalar.activation(
                    ebias[:, c * 4 : (c + 1) * 4], bp[:, 0:16:4], Exp,
                )
                for j in range(4):
                    mt = c * 4 + j
                    nc.vector.tensor_scalar_mul(
                        valaug[:, mt, 0:H],
                        vtr[:, j * H : (j + 1) * H],
                        ebias[:, mt : mt + 1],
                    )
                    nc.vector.tensor_copy(
                        valaug[:, mt, H : H + 1], ebias[:, mt : mt + 1]
                    )

                # fused attention for everything unlocked by chunk c
                for tq, tiles in _attn_events(c):
                    tsl = slice(tq * 512, (tq + 1) * 512)
                    total = len(tiles)
                    for gi in range(0, total, 2):
                        grp = tiles[gi : gi + 2]
                        sp2 = psc.tile([P, 2, 512], f32, name="sp", tag="sc")
                        for i, (mt, rel) in enumerate(grp):
                            msl = slice(mt * P, (mt + 1) * P)
                            nc.tensor.matmul(
                                sp2[:, i, :], K2[:, msl], q2T[:, tsl],
                                start=True, stop=True,
                            )
                        ex2 = expool.tile([P, 2, 512], f32r, name="ex",
                                          tag="ex")
                        nc.scalar.activation(ex2[:], sp2[:], Exp)
                        for i, (mt, rel) in enumerate(grp):
                            if rel is not None:
                                # zero masked entries on the idle Pool
                                # engine (f32r ALU output is rounded, so
                                # the attnval matmul accepts it)
                                nc.gpsimd.tensor_mul(
                                    ex2[:, i, :], ex2[:, i, :],
                                    dmask[:, rel, :],
                                )
                        pend.append((c, tq, grp, ex2, total))
                        if len(pend) > LAGG:
                            pop_group()
            while pend:
                pop_group()
            emit_output(NTQ - 1)

    nc.compile()
    return nc


def _get_module():
    if "nc" not in _CACHE:
        _CACHE["nc"] = _build_module()
    return _CACHE["nc"]


def _make_in_maps(inputs):
    inp = np.asarray(inputs["inp_data"], dtype=np.float32)
    Wq = np.asarray(inputs["Wq"], dtype=np.float32)
    bq = np.asarray(inputs["bq"], dtype=np.float32)
    Wk = np.asarray(inputs["Wk"], dtype=np.float32)
    bk = np.asarray(inputs["bk"], dtype=np.float32)
    Wv = np.asarray(inputs["Wv"], dtype=np.float32)
    bv = np.asarray(inputs["bv"], dtype=np.float32)
    Wr = np.asarray(inputs["Wr"], dtype=np.float32)
    br = np.asarray(inputs["br"], dtype=np.float32)
    u = np.asarray(inputs["u"], dtype=np.float32)
    v = np.asarray(inputs["v"], dtype=np.float32)

    if "pe0" not in _CACHE:
        _CACHE["pe0"] = np.ascontiguousarray(_pe0T().astype(np.float16))
    us, vs = np.float32(u.sum()), np.float32(v.sum())
    sm = np.zeros((2 * H, 12), dtype=np.float32)
    sm[:, 0] = np.concatenate([bv, bk])       # bkv, chunks 0-3 ([v; k])
    sm[:, 1] = np.concatenate([bk, bv])       # bkv, chunks 4-7 ([k; v])
    sm[:, 2] = np.concatenate([bq, bq])       # bqq
    sm[0:H, 3:7], sm[H:, 3:7] = vs, us        # uv col, chunks 0-3 [r; k]
    sm[0:H, 7:11], sm[H:, 7:11] = us, vs      # uv col, chunks 4-7 [k; r]
    sm[0:H, 11] = br                          # br on both pair halves
    sm[H:, 11] = br
    common = {
        "pe0": _CACHE["pe0"],
        "wrot": np.ascontiguousarray(_rotate_wr(Wr).astype(np.float16)),
        "wkva": np.ascontiguousarray(np.concatenate([Wv, Wk], axis=1)),
        "wkvb": np.ascontiguousarray(np.concatenate([Wk, Wv], axis=1)),
        "wqq": np.ascontiguousarray(np.concatenate([Wq, Wq], axis=1)),
        "smalls": sm,
    }
    in_maps = []
    for b in range(NCORES):
        m = {"xT": np.ascontiguousarray(inp[:, b, :].T)}
        m.update(common)
        in_maps.append(m)
    return in_maps


def _run(in_maps, trace=False):
    from concourse.bass_utils import run_bass_kernel_spmd

    nc = _get_module()
    return run_bass_kernel_spmd(
        nc, in_maps, core_ids=list(range(NCORES)), trace=trace
    )


def _timed_run(in_maps, iters=5, reps=1):
    """Replicates bass2jax.run_bass_via_pjrt's multi-core path, but keeps the
    jitted callable and device-resident inputs so repeated executions can be
    wall-clock timed (no NTFF profiling is available through the axon client).
    """
    import time

    import jax
    import concourse.mybir as mybir
    from concourse.bass2jax import (
        _bass_exec_p,
        install_neuronx_cc_hook,
        partition_id_tensor,
    )
    from jax.experimental.shard_map import shard_map
    from jax.sharding import Mesh, NamedSharding, PartitionSpec

    nc = _get_module()
    install_neuronx_cc_hook()
    partition_name = nc.partition_id_tensor.name if nc.partition_id_tensor else None

    in_names, out_names, out_avals, zero_shapes = [], [], [], []
    for alloc in nc.m.functions[0].allocations:
        if not isinstance(alloc, mybir.MemoryLocationSet):
            continue
        name = alloc.memorylocations[0].name
        if alloc.kind == "ExternalInput":
            if name != partition_name:
                in_names.append(name)
        elif alloc.kind == "ExternalOutput":
            out_names.append(name)
            shape = tuple(alloc.tensor_shape)
            dtype = mybir.dt.np(alloc.dtype)
            out_avals.append(jax.core.ShapedArray(shape, dtype))
            zero_shapes.append((shape, dtype))
    n_params = len(in_names)
    all_names = in_names + out_names
    if partition_name is not None:
        all_names = all_names + [partition_name]
    donate = tuple(range(n_params, n_params + len(out_names)))

    def _body(*args):
        operands = list(args)
        if partition_name is not None:
            operands.append(partition_id_tensor())
        outs = _bass_exec_p.bind(
            *operands,
            out_avals=tuple(out_avals),
            in_names=tuple(all_names),
            out_names=tuple(out_names),
            lowering_input_output_aliases=(),
            sim_require_finite=True,
            sim_require_nnan=True,
            nc=nc,
        )
        return tuple(outs)

    devices = jax.devices()[:NCORES]
    mesh = Mesh(np.asarray(devices), ("core",))
    spec = PartitionSpec("core")
    in_specs = (spec,) * (n_params + len(out_names))
    sharded = jax.jit(
        shard_map(
            _body, mesh=mesh, in_specs=in_specs,
            out_specs=(spec,) * len(out_names), check_rep=False,
        ),
        donate_argnums=donate,
        keep_unused=True,
    )
    sharding = NamedSharding(mesh, spec)
    concat_in = [
        jax.device_put(
            np.concatenate([in_maps[c][nm] for c in range(NCORES)], axis=0), sharding
        )
        for nm in in_names
    ]

    def zeros():
        return [
            jax.device_put(np.zeros((NCORES * s[0], *s[1:]), d), sharding)
            for (s, d) in zero_shapes
        ]

    out = sharded(*concat_in, *zeros())
    jax.block_until_ready(out)
    times = []
    for _ in range(iters):
        zs = zeros()
        jax.block_until_ready(zs)
        t0 = time.perf_counter()
        out = sharded(*concat_in, *zs)
        jax.block_until_ready(out)
        times.append(time.perf_counter() - t0)
    results = {
        nm: np.asarray(out[i]).reshape(NCORES, *out_avals[i].shape)
        for i, nm in enumerate(out_names)
    }
    return results, times


def kernel(**inputs) -> np.ndarray:
    in_maps = _make_in_maps(inputs)
    res = _run(in_maps, trace=False)
    out = np.stack([res.results[b]["out"] for b in range(NCORES)], axis=1)
    return np.ascontiguousarray(out.astype(np.float32))
